# revision 20
# baseline (speedup 1.0000x reference)
"""Trainium2 Bass kernel for nn_AutoEncoder (6-layer GCN autoencoder).

Strategy (8 NeuronCores, SPMD):
  - Destination nodes sharded across cores (6250/core, padded to 6272).
  - Node features kept pre-scaled by deg^-1/2 ("hs") and replicated on every
    core in a padded [8*6272, F] layout (per-layer AllGather, bf16 except the
    64-wide bottleneck layer which must stay f32 for the 256B-row DMA-gather
    constraint).
  - Per layer: dma_gather of hs[src] for this core's edges (edge list sorted
    by local dst, split by int16-index halves), segment-sum via one-hot
    matmuls accumulated in PSUM (128-dst windows); self-loop contributions
    enter the same PSUM banks as PE transposes of the resident own-shard
    tile. Aggregate is scaled by deg^-1/2[dst] on eviction, W matmul in f32,
    then raw y is PE-transposed to node-major while the BatchNorm (sum,
    sumsq) AllReduce is in flight; BN + ReLU + deg^-1/2 rescale are applied
    post-AllReduce in node-major via rank-1 broadcast tiles.
  - The GCN bias b is skipped (training-mode BatchNorm makes any per-feature
    constant shift a no-op).
"""

import sys

sys.path.insert(0, "/opt/trn_rl_repo")

import numpy as np

N = 50000
E = 800000
F_IN = 128
EPS = 1e-5
NC = 8
SH = 6250  # real dst nodes per core
SHP = 6272  # padded (49 * 128)
NP = NC * SHP  # 50176 rows in the padded replicated node table
HALF = NP // 2  # 25088 (< int16 max) rows per gather table half
WIN = 128  # dst window = psum column band
NWIN = SHP // WIN  # 49
NSB = 13  # psum superblocks: 12 x 512 + 1 x 128
CHUNK = 32  # gather chunk size in K-tiles
NQ = 4  # SWDGE queues (round-robin; each runs on its own Q7 core pair)
DIMS = [(128, 128), (128, 128), (128, 64), (64, 128), (128, 128), (128, 128)]
RELU = [True, True, False, True, True, False]
TBLW = 128  # table row width (64-wide bottleneck zero-padded to 128, fp16)

import os as _os
DEBUG_NL = int(_os.environ.get("DEBUG_NL", "6"))  # layers to run (debug)
DEBUG_PREBN = int(_os.environ.get("DEBUG_PREBN", "0"))
DEBUG_REPS = int(_os.environ.get("DEBUG_REPS", "0"))
TRACE = False  # set by test.py for profiling runs
TRACE_KW = {}
LAST_RESULT = None  # BassKernelResults of the last run (for test.py)


def _prep_edges(src_remap, dstl):
    """Per-core edge prep: sort by dst; per-(window, half) edge lists."""
    order = np.argsort(dstl, kind="stable")
    dstl = dstl[order]
    srcr = src_remap[order]
    half = (srcr >= HALF).astype(np.int64)
    w = dstl // WIN
    rel = dstl - w * WIN
    ed = [[None, None] for _ in range(NWIN)]
    for wi in range(NWIN):
        m = w == wi
        for h in (0, 1):
            mh = m & (half == h)
            ed[wi][h] = (srcr[mh] - h * HALF, rel[mh])
    return ed


def _build_core_tables(ed, tiles):
    """Pack per-core edge lists into padded tile streams (per half)."""
    out = []
    for h in (0, 1):
        T = sum(tiles[w][h] for w in range(NWIN))
        gidx = np.zeros(T * 128, np.int16)
        drel = np.full(T * 128, -1.0, np.float32)
        t = 0
        for w in range(NWIN):
            g, r = ed[w][h]
            nt = tiles[w][h]
            assert len(g) <= nt * 128
            base = t * 128
            gidx[base : base + len(g)] = g.astype(np.int16)
            drel[base : base + len(g)] = r.astype(np.float32)
            t += nt
        out.append((gidx, drel))
    return out


def _wrap_idx(gidx, chunks):
    """int16 indices -> [128, total/16] wrapped per chunk, tiled 8x."""
    total_cols = len(gidx) // 16
    arr = np.zeros((16, total_cols), np.int16)
    col = 0
    for t0, nt in chunks:
        cidx = gidx[t0 * 128 : (t0 + nt) * 128]
        ncol = len(cidx) // 16
        arr[:, col : col + ncol] = cidx.reshape(ncol, 16).T
        col += ncol
    assert col == total_cols
    return np.tile(arr, (8, 1)).copy()


def _chunks_of(T):
    out = []
    t = 0
    while t < T:
        nt = min(CHUNK, T - t)
        out.append((t, nt))
        t += nt
    return out


def _build_program(tiles, t0s, TA, TB):
    from concourse import bacc, mybir, tile

    FP32 = mybir.dt.float32
    BF16 = mybir.dt.float16  # "BF16" alias now fp16: same perf, 8x finer mantissa
    I16 = mybir.dt.int16
    AX = mybir.AxisListType.X
    OP = mybir.AluOpType
    ACTF = mybir.ActivationFunctionType

    nc = bacc.Bacc(
        None,
        num_devices=NC,
        target_bir_lowering=False,
        debug=False,
        num_swdge_queues=NQ,
    )

    # ---- parameters ----
    HS0DT = BF16
    hs0_d = nc.declare_dram_parameter("hs0", [NP, F_IN], HS0DT, isOutput=False)
    hs0own_d = nc.declare_dram_parameter(
        "hs0own", [128, NWIN, F_IN], HS0DT, isOutput=False
    )
    idx_d = [
        nc.declare_dram_parameter("idxA", [128, TA * 8], I16, isOutput=False),
        nc.declare_dram_parameter("idxB", [128, TB * 8], I16, isOutput=False),
    ]
    stiles_d = [
        nc.declare_dram_parameter("stilesA", [TA, 128, 128], BF16, isOutput=False),
        nc.declare_dram_parameter("stilesB", [TB, 128, 128], BF16, isOutput=False),
    ]
    dinv_d = nc.declare_dram_parameter("dinvT", [128, SHP], FP32, isOutput=False)
    ident_d = nc.declare_dram_parameter("ident", [128, 128], FP32, isOutput=False)
    W_d = [
        nc.declare_dram_parameter(f"W{j}", list(DIMS[j]), FP32, isOutput=False)
        for j in range(6)
    ]
    gb_d = [
        nc.declare_dram_parameter(f"gb{j}", [128, 2], FP32, isOutput=False)
        for j in range(6)
    ]
    ones_d = nc.declare_dram_parameter("ones", [1, 128], FP32, isOutput=False)
    dinvnb_d = nc.declare_dram_parameter("dinvNB", [128, NWIN], FP32, isOutput=False)
    out_d = nc.declare_dram_parameter("out", [128, SHP], FP32, isOutput=True)

    # ---- internal DRAM: collective bounce buffers ----
    # table for layer j+1 is ag_out[j]; all fp16, 128-wide (64 zero-padded)
    ag_dt = [BF16 for j in range(5)]
    ag_in = [nc.dram_tensor(f"ag_in{j}", [SHP, TBLW], ag_dt[j]) for j in range(5)]
    ag_out = [
        nc.dram_tensor(f"ag_out{j}", [NP, TBLW], ag_dt[j], addr_space="Shared")
        for j in range(5)
    ]
    ar_in = [nc.dram_tensor(f"ar_in{j}", [128, 2], FP32) for j in range(6)]
    ar_out = [
        nc.dram_tensor(f"ar_out{j}", [128, 2], FP32, addr_space="Shared")
        for j in range(6)
    ]

    tbls = [hs0_d] + ag_out

    chunks = [_chunks_of(TA), _chunks_of(TB)]
    idx_col0 = [[], []]
    for h in (0, 1):
        c = 0
        for _, nt in chunks[h]:
            idx_col0[h].append(c)
            c += nt * 8

    with tile.TileContext(nc) as tc:
        with (
            tc.tile_pool(name="res", bufs=1) as res,
            tc.tile_pool(name="msg", bufs=8) as msgp,
            tc.tile_pool(name="sp", bufs=4) as sp,
            tc.tile_pool(name="small", bufs=2) as small,
            tc.tile_pool(name="big", bufs=1) as big,
            tc.tile_pool(name="hx", bufs=2) as hxp,
            tc.tile_pool(name="agg_ps", bufs=2, space="PSUM") as aggp,
            tc.tile_pool(name="y_ps", bufs=2, space="PSUM") as yp,
            tc.tile_pool(name="tr_ps", bufs=2, space="PSUM") as trp,
        ):
            # ---- resident loads ----
            idx_t = [res.tile([128, TA * 8], I16, name="idxA")]
            nc.sync.dma_start(idx_t[0][:], idx_d[0][:])
            idx_t.append(res.tile([128, TB * 8], I16, name="idxB"))
            nc.sync.dma_start(idx_t[1][:], idx_d[1][:])
            ident_f = res.tile([128, 128], FP32, name="identf")
            nc.sync.dma_start(ident_f[:], ident_d[:])
            ident_b = res.tile([128, 128], BF16, name="identb")
            nc.vector.tensor_copy(ident_b[:], ident_f[:])
            ones_t = res.tile([1, 128], FP32, name="ones")
            nc.sync.dma_start(ones_t[:], ones_d[:])
            dinvnb_t = res.tile([128, NWIN], FP32, name="dinvnb")
            nc.sync.dma_start(dinvnb_t[:], dinvnb_d[:])
            W_t = []
            for j in range(6):
                wt = res.tile(list(DIMS[j]), FP32, name=f"W{j}")
                nc.sync.dma_start(wt[:], W_d[j][:])
                W_t.append(wt)
            gb_t = []
            for j in range(6):
                gt = res.tile([128, 2], FP32, name=f"gb{j}")
                nc.sync.dma_start(gt[:], gb_d[j][:])
                gb_t.append(gt)

            # own-shard tile (node-major; source for self-loop transposes)
            hprev = hxp.tile([128, NWIN, F_IN], HS0DT, tag="hx", name="hs0own")
            nc.sync.dma_start(hprev[:], hs0own_d[:])
            hprev_dt = HS0DT

            inv_n = 1.0 / float(N)
            qrr = [0]  # gather queue round-robin counter

            def bn_vec(j, fo, arr_tile):
                """mean/var -> (scale, shift) columns in a [128, 6] tile."""
                vec = small.tile([128, 6], FP32, tag="bnvec", name="vec")
                nc.vector.tensor_scalar(
                    out=vec[0:fo, 0:1], in0=arr_tile[0:fo, 0:1],
                    scalar1=inv_n, scalar2=None, op0=OP.mult,
                )
                nc.vector.tensor_scalar(
                    out=vec[0:fo, 1:2], in0=arr_tile[0:fo, 1:2],
                    scalar1=inv_n, scalar2=None, op0=OP.mult,
                )
                nc.vector.tensor_tensor(
                    vec[0:fo, 2:3], vec[0:fo, 0:1], vec[0:fo, 0:1], op=OP.mult
                )
                nc.vector.tensor_tensor(
                    vec[0:fo, 2:3], vec[0:fo, 1:2], vec[0:fo, 2:3],
                    op=OP.subtract,
                )
                nc.vector.tensor_scalar(
                    out=vec[0:fo, 2:3], in0=vec[0:fo, 2:3],
                    scalar1=float(EPS), scalar2=None, op0=OP.add,
                )
                nc.vector.reciprocal(vec[0:fo, 3:4], vec[0:fo, 2:3])
                nc.scalar.activation(vec[0:fo, 3:4], vec[0:fo, 3:4], ACTF.Sqrt)
                nc.vector.tensor_tensor(
                    vec[0:fo, 4:5], gb_t[j][0:fo, 0:1], vec[0:fo, 3:4],
                    op=OP.mult,
                )
                nc.vector.tensor_tensor(
                    vec[0:fo, 5:6], vec[0:fo, 0:1], vec[0:fo, 4:5], op=OP.mult
                )
                nc.vector.tensor_tensor(
                    vec[0:fo, 5:6], gb_t[j][0:fo, 1:2], vec[0:fo, 5:6],
                    op=OP.subtract,
                )
                return vec

            for j in range(DEBUG_NL):
                fi, fo = DIMS[j]
                tbl = tbls[j]
                MDT = BF16
                ident_in = ident_b if hprev_dt == BF16 else ident_f

                cur_chunk = [-1, -1]
                msg_tiles = [None, None]
                s_tiles = {}

                def ensure_chunk(h, t):
                    k = 0
                    while not (
                        chunks[h][k][0] <= t < chunks[h][k][0] + chunks[h][k][1]
                    ):
                        k += 1
                    if cur_chunk[h] == k:
                        return
                    cur_chunk[h] = k
                    t0c, ntc = chunks[h][k]
                    mt = msgp.tile([128, ntc, TBLW], MDT, tag="msg", name="msg")
                    nc.gpsimd.dma_gather(
                        out_ap=mt[:],
                        in_ap=tbl[h * HALF : (h + 1) * HALF, :],
                        idxs_ap=idx_t[h][
                            :, idx_col0[h][k] : idx_col0[h][k] + ntc * 8
                        ],
                        num_idxs=ntc * 128,
                        num_idxs_reg=ntc * 128,
                        elem_size=TBLW,
                        single_packet=False,
                        queue_num=qrr[0] % NQ,
                    )
                    qrr[0] += 1
                    msg_tiles[h] = (t0c, mt)

                y_sb = big.tile([128, SHP], FP32, tag="ysb", name="ysb")
                sumP = small.tile([128, NSB], FP32, tag="sumP", name="sumP")
                sqP = small.tile([128, NSB], FP32, tag="sqP", name="sqP")
                junk = small.tile([128, 512], FP32, tag="junk", name="junk")

                for sb in range(NSB):
                    nsb = 512 if sb < 12 else 128
                    wlist = list(range(sb * 4, min(sb * 4 + 4, NWIN)))
                    # tile sequence; (w, None, None) = self-loop transpose
                    seq = []
                    for w in wlist:
                        seq.append((w, None, None))
                        for h in (0, 1):
                            for t in range(t0s[w][h], t0s[w][h] + tiles[w][h]):
                                seq.append((w, h, t))
                    agg = aggp.tile([128, 512], FP32, tag="agg", name="agg")
                    for i, (w, h, t) in enumerate(seq):
                        woff = (w % 4) * 128
                        first = i == 0
                        last = i == len(seq) - 1
                        if h is None:
                            # self-loop: agg[f, d] += hprev[d, f], done as a
                            # regular matmul with identity moving operand
                            # (lhsT=hprev) so fp16 input can hit f32 PSUM.
                            nc.tensor.matmul(
                                agg[0:fi, woff : woff + 128],
                                hprev[:, w, 0:fi],
                                ident_in[:],
                                start=first,
                                stop=last,
                            )
                            continue
                        ensure_chunk(h, t)
                        t0c, mt = msg_tiles[h]
                        if (w, h) not in s_tiles:
                            G = tiles[w][h]
                            st = sp.tile([128, G, 128], MDT, tag="S", name="S")
                            nc.sync.dma_start(
                                st[:],
                                stiles_d[h][
                                    t0s[w][h] : t0s[w][h] + G
                                ].rearrange("g p d -> p g d"),
                            )
                            s_tiles[(w, h)] = (st, t0s[w][h])
                        st, st_t0 = s_tiles[(w, h)]
                        nc.tensor.matmul(
                            agg[0:fi, woff : woff + 128],
                            mt[:, t - t0c, 0:fi],
                            st[:, t - st_t0, :],
                            start=first,
                            stop=last,
                        )
                    # evict + dinv[dst] scale
                    dv = small.tile([128, 512], FP32, tag="dinv", name="dv")
                    nc.sync.dma_start(
                        dv[:, 0:nsb], dinv_d[:, sb * 512 : sb * 512 + nsb]
                    )
                    rawT = small.tile([128, 512], FP32, tag="rawT", name="rawT")
                    nc.vector.tensor_tensor(
                        rawT[0:fi, 0:nsb],
                        agg[0:fi, 0:nsb],
                        dv[0:fi, 0:nsb],
                        op=OP.mult,
                    )
                    # W matmul (f32)
                    y_ps = yp.tile([128, 512], FP32, tag="yps", name="yps")
                    nc.tensor.matmul(
                        y_ps[0:fo, 0:nsb],
                        W_t[j][:],
                        rawT[0:fi, 0:nsb],
                        start=True,
                        stop=True,
                    )
                    # copy to y_sb + stats over valid columns
                    nv = 512 if sb < 12 else 106
                    c0 = sb * 512
                    nc.scalar.activation(
                        y_sb[0:fo, c0 : c0 + nv],
                        y_ps[0:fo, 0:nv],
                        ACTF.Copy,
                        accum_out=sumP[0:fo, sb : sb + 1],
                    )
                    if sb == 12:
                        nc.scalar.activation(
                            y_sb[0:fo, c0 + 106 : c0 + 128],
                            y_ps[0:fo, 106:128],
                            ACTF.Copy,
                        )
                    nc.scalar.activation(
                        junk[0:fo, 0:nv],
                        y_ps[0:fo, 0:nv],
                        ACTF.Square,
                        accum_out=sqP[0:fo, sb : sb + 1],
                    )

                # ---- kick BN stats all-reduce ----
                stats = small.tile([128, 2], FP32, tag="stats", name="stats")
                nc.vector.memset(stats[:], 0.0)
                nc.vector.reduce_sum(stats[0:fo, 0:1], sumP[0:fo, :], axis=AX)
                nc.vector.reduce_sum(stats[0:fo, 1:2], sqP[0:fo, :], axis=AX)
                nc.sync.dma_start(ar_in[j][:], stats[:])
                nc.gpsimd.collective_compute(
                    "AllReduce",
                    OP.add,
                    replica_groups=[list(range(NC))],
                    ins=[ar_in[j][:]],
                    outs=[ar_out[j][:]],
                )

                if j == DEBUG_NL - 1 and j != 5:
                    pass  # fall through; debug dump happens below
                if j == 5:
                    # final layer: BN in feat-major via ACT, DMA out
                    arr = small.tile([128, 2], FP32, tag="arr", name="arr")
                    nc.sync.dma_start(arr[:], ar_out[j][:])
                    vec = bn_vec(j, fo, arr)
                    for sb in range(NSB):
                        nsb = 512 if sb < 12 else 128
                        c0 = sb * 512
                        nc.scalar.activation(
                            y_sb[0:fo, c0 : c0 + nsb],
                            y_sb[0:fo, c0 : c0 + nsb],
                            ACTF.Identity,
                            bias=vec[0:fo, 5:6],
                            scale=vec[0:fo, 4:5],
                        )
                    nc.sync.dma_start(out_d[:], y_sb[:])
                    continue

                # ---- transpose raw y to node-major (overlaps the AR) ----
                hnext = hxp.tile([128, NWIN, TBLW], ag_dt[j], tag="hx", name="hnext")
                if fo < TBLW:
                    nc.vector.memset(hnext[:, :, fo:TBLW], 0.0)
                for b0 in range(0, NWIN, 4):
                    nb = min(4, NWIN - b0)
                    tr4 = trp.tile([128, 512], FP32, tag="tr", name="tr4")
                    for bi in range(nb):
                        b = b0 + bi
                        nc.tensor.matmul(
                            tr4[0:128, bi * fo : bi * fo + fo],
                            y_sb[0:fo, b * 128 : (b + 1) * 128],
                            ident_f[0:fo, 0:fo],
                            is_transpose=True,
                            start=True,
                            stop=True,
                        )
                    nc.vector.tensor_copy(
                        hnext[:, b0 : b0 + nb, 0:fo],
                        tr4[:, 0 : nb * fo].rearrange("p (b f) -> p b f", f=fo),
                    )

                # ---- AR result -> scale/shift -> rank-1 broadcast tiles ----
                arr = small.tile([128, 2], FP32, tag="arr", name="arr")
                nc.sync.dma_start(arr[:], ar_out[j][:])
                vec = bn_vec(j, fo, arr)
                # transpose scale/shift columns to rows, then rank-1 bcast
                scale_rep = small.tile(
                    [128, 128], ag_dt[j], tag="srep", name="scale_rep"
                )
                shift_rep = small.tile(
                    [128, 128], ag_dt[j], tag="hrep", name="shift_rep"
                )
                for col, rep in ((4, scale_rep), (5, shift_rep)):
                    vt_ps = trp.tile([128, 128], FP32, tag="tr", name="vtps")
                    nc.tensor.matmul(
                        vt_ps[0:1, 0:fo],
                        vec[0:fo, col : col + 1],
                        ident_f[0:fo, 0:fo],
                        is_transpose=True,
                        start=True,
                        stop=True,
                    )
                    vrow = small.tile([1, 128], FP32, tag="vrow", name="vrow")
                    nc.vector.tensor_copy(vrow[:, 0:fo], vt_ps[0:1, 0:fo])
                    rep_ps = trp.tile([128, 128], FP32, tag="tr", name="repps")
                    nc.tensor.matmul(
                        rep_ps[:, 0:fo], ones_t[:], vrow[0:1, 0:fo],
                        start=True, stop=True,
                    )
                    nc.vector.tensor_copy(rep[:, 0:fo], rep_ps[:, 0:fo])

                if DEBUG_REPS and j == DEBUG_NL - 1:
                    nc.sync.dma_start(out_d[:, 0:128], scale_rep[:])
                    nc.sync.dma_start(out_d[:, 128:256], shift_rep[:])
                    nc.sync.dma_start(out_d[:, 256:262], vec[:, 0:6])
                    continue
                # ---- BN apply (+ReLU) in node-major, in place ----
                for b in range(NWIN if not (DEBUG_PREBN and j == DEBUG_NL - 1) else 0):
                    blk = hnext[:, b, 0:fo]
                    nc.vector.tensor_tensor(
                        blk, blk, scale_rep[:, 0:fo], op=OP.mult
                    )
                    nc.vector.tensor_tensor(
                        blk, blk, shift_rep[:, 0:fo], op=OP.add
                    )
                    if RELU[j]:
                        nc.vector.tensor_scalar(
                            out=blk, in0=blk,
                            scalar1=dinvnb_t[:, b : b + 1], scalar2=0.0,
                            op0=OP.mult, op1=OP.max,
                        )
                    else:
                        nc.vector.tensor_scalar(
                            out=blk, in0=blk,
                            scalar1=dinvnb_t[:, b : b + 1], scalar2=None,
                            op0=OP.mult,
                        )

                if j == DEBUG_NL - 1:
                    nc.sync.dma_start(
                        out_d[:, 0 : NWIN * fo].rearrange(
                            "p (b f) -> p b f", f=fo
                        ),
                        hnext[:, :, 0:fo],
                    )
                    continue
                nc.sync.dma_start(
                    ag_in[j][:].rearrange("(b p) f -> p b f", p=128), hnext[:]
                )
                nc.gpsimd.collective_compute(
                    "AllGather",
                    OP.bypass,
                    replica_groups=[list(range(NC))],
                    ins=[ag_in[j][:]],
                    outs=[ag_out[j][:]],
                )
                hprev = hnext
                hprev_dt = ag_dt[j]

    nc.compile()
    return nc


def kernel(x, edge_index, **params):
    global LAST_RESULT
    import ml_dtypes

    from concourse.bass_utils import run_bass_kernel_spmd

    x = np.asarray(x, np.float32)
    edge_index = np.asarray(edge_index, np.int64)
    src_all = edge_index[0]
    dst_all = edge_index[1]

    deg = (np.bincount(dst_all, minlength=N) + 1.0).astype(np.float32)
    dinv = (1.0 / np.sqrt(deg)).astype(np.float32)

    hs0 = np.zeros((NP, F_IN), np.float32)
    xs = x * dinv[:, None]
    for c in range(NC):
        hs0[c * SHP : c * SHP + SH] = xs[c * SH : (c + 1) * SH]
    hs0_bf = hs0.astype(np.float16)

    remap = (src_all // SH) * SHP + (src_all % SH)

    eds = []
    for c in range(NC):
        m = (dst_all >= c * SH) & (dst_all < (c + 1) * SH)
        dstl = dst_all[m] - c * SH
        srcr = remap[m]
        eds.append(_prep_edges(srcr, dstl))

    tiles = [[0, 0] for _ in range(NWIN)]
    for w in range(NWIN):
        for h in (0, 1):
            mx = max(len(eds[c][w][h][0]) for c in range(NC))
            tiles[w][h] = -(-mx // 128) if mx else 0
    t0s = [[0, 0] for _ in range(NWIN)]
    ta = tb = 0
    for w in range(NWIN):
        t0s[w][0] = ta
        ta += tiles[w][0]
        t0s[w][1] = tb
        tb += tiles[w][1]
    TA, TB = ta, tb

    chunksA = _chunks_of(TA)
    chunksB = _chunks_of(TB)

    def _build_stiles(drel, T):
        """One-hot S tiles [T, 128, 128] fp16 from rel-dst stream (pad=-1)."""
        s = np.zeros((T * 128, 128), np.float16)
        rel = drel.astype(np.int64)
        valid = np.nonzero(rel >= 0)[0]
        s[valid, rel[valid]] = 1.0
        return s.reshape(T, 128, 128)

    in_maps = []
    for c in range(NC):
        (gA, dA), (gB, dB) = _build_core_tables(eds[c], tiles)
        dinvT = np.zeros(SHP, np.float32)
        dinvT[:SH] = dinv[c * SH : (c + 1) * SH]
        # own shard in [128, NWIN, F] node-major layout: [p, b, :] = node b*128+p
        own = hs0_bf[c * SHP : (c + 1) * SHP].reshape(NWIN, 128, F_IN)
        im = {
            "hs0": hs0_bf,
            "hs0own": np.ascontiguousarray(own.transpose(1, 0, 2)),
            "idxA": _wrap_idx(gA, chunksA),
            "idxB": _wrap_idx(gB, chunksB),
            "stilesA": _build_stiles(dA, TA),
            "stilesB": _build_stiles(dB, TB),
            "dinvT": np.broadcast_to(dinvT, (128, SHP)).copy(),
            "ident": np.eye(128, dtype=np.float32),
            "ones": np.ones((1, 128), np.float32),
            "dinvNB": np.ascontiguousarray(
                dinvT.reshape(NWIN, 128).T
            ),
        }
        for j in range(6):
            im[f"W{j}"] = np.asarray(params[f"W{j}"], np.float32)
            gb = np.zeros((128, 2), np.float32)
            fo = DIMS[j][1]
            gb[:fo, 0] = np.asarray(params[f"g{j}"], np.float32)
            gb[:fo, 1] = np.asarray(params[f"be{j}"], np.float32)
            im[f"gb{j}"] = gb
        in_maps.append(im)

    nc = _build_program(tiles, t0s, TA, TB)
    res = run_bass_kernel_spmd(
        nc,
        in_maps,
        core_ids=list(range(NC)),
        trace=TRACE,
        **TRACE_KW,
    )
    LAST_RESULT = res

    out = np.empty((N, F_IN), np.float32)
    for c in range(NC):
        out[c * SH : (c + 1) * SH] = res.results[c]["out"].T[:SH]
    return out



# revision 23
# speedup vs baseline: 1.0965x; 1.0965x over previous
"""Trainium2 Bass kernel for nn_AutoEncoder (6-layer GCN autoencoder).

Strategy (8 NeuronCores, SPMD):
  - Destination nodes sharded across cores (6250/core, padded to 6272).
  - Node features kept pre-scaled by deg^-1/2 ("hs") and replicated on every
    core in a padded [8*6272, F] layout (per-layer AllGather, bf16 except the
    64-wide bottleneck layer which must stay f32 for the 256B-row DMA-gather
    constraint).
  - Per layer: dma_gather of hs[src] for this core's edges (edge list sorted
    by local dst, split by int16-index halves), segment-sum via one-hot
    matmuls accumulated in PSUM (128-dst windows); self-loop contributions
    enter the same PSUM banks as PE transposes of the resident own-shard
    tile. Aggregate is scaled by deg^-1/2[dst] on eviction, W matmul in f32,
    then raw y is PE-transposed to node-major while the BatchNorm (sum,
    sumsq) AllReduce is in flight; BN + ReLU + deg^-1/2 rescale are applied
    post-AllReduce in node-major via rank-1 broadcast tiles.
  - The GCN bias b is skipped (training-mode BatchNorm makes any per-feature
    constant shift a no-op).
"""

import sys

sys.path.insert(0, "/opt/trn_rl_repo")

import numpy as np

N = 50000
E = 800000
F_IN = 128
EPS = 1e-5
NC = 8
SH = 6250  # real dst nodes per core
SHP = 6272  # padded (49 * 128)
NP = NC * SHP  # 50176 rows in the padded replicated node table
HALF = NP // 2  # 25088 (< int16 max) rows per gather table half
WIN = 128  # dst window = psum column band
NWIN = SHP // WIN  # 49
NSB = 13  # psum superblocks: 12 x 512 + 1 x 128
CHUNK = 32  # gather chunk size in K-tiles
NQ = 4  # SWDGE queues (round-robin; each runs on its own Q7 core pair)
DIMS = [(128, 128), (128, 128), (128, 64), (64, 128), (128, 128), (128, 128)]
RELU = [True, True, False, True, True, False]
TBLW = 128  # table row width (64-wide bottleneck zero-padded to 128, fp16)

import os as _os
DEBUG_NL = int(_os.environ.get("DEBUG_NL", "6"))  # layers to run (debug)
DEBUG_PREBN = int(_os.environ.get("DEBUG_PREBN", "0"))
DEBUG_REPS = int(_os.environ.get("DEBUG_REPS", "0"))
TRACE = False  # set by test.py for profiling runs
TRACE_KW = {}
LAST_RESULT = None  # BassKernelResults of the last run (for test.py)


def _prep_edges(src_remap, dstl):
    """Per-core edge prep: sort by dst; per-(window, half) edge lists."""
    order = np.argsort(dstl, kind="stable")
    dstl = dstl[order]
    srcr = src_remap[order]
    half = (srcr >= HALF).astype(np.int64)
    w = dstl // WIN
    rel = dstl - w * WIN
    ed = [[None, None] for _ in range(NWIN)]
    for wi in range(NWIN):
        m = w == wi
        for h in (0, 1):
            mh = m & (half == h)
            ed[wi][h] = (srcr[mh] - h * HALF, rel[mh])
    return ed


def _build_core_tables(ed, tiles):
    """Pack per-core edge lists into padded tile streams (per half)."""
    out = []
    for h in (0, 1):
        T = sum(tiles[w][h] for w in range(NWIN))
        gidx = np.zeros(T * 128, np.int16)
        drel = np.full(T * 128, -1.0, np.float32)
        t = 0
        for w in range(NWIN):
            g, r = ed[w][h]
            nt = tiles[w][h]
            assert len(g) <= nt * 128
            base = t * 128
            gidx[base : base + len(g)] = g.astype(np.int16)
            drel[base : base + len(g)] = r.astype(np.float32)
            t += nt
        out.append((gidx, drel))
    return out


def _wrap_idx(gidx, chunks):
    """int16 indices -> [128, total/16] wrapped per chunk, tiled 8x."""
    total_cols = len(gidx) // 16
    arr = np.zeros((16, total_cols), np.int16)
    col = 0
    for t0, nt in chunks:
        cidx = gidx[t0 * 128 : (t0 + nt) * 128]
        ncol = len(cidx) // 16
        arr[:, col : col + ncol] = cidx.reshape(ncol, 16).T
        col += ncol
    assert col == total_cols
    return np.tile(arr, (8, 1)).copy()


def _chunks_of(T):
    out = []
    t = 0
    while t < T:
        nt = min(CHUNK, T - t)
        out.append((t, nt))
        t += nt
    return out


def _build_program(tiles, t0s, TA, TB):
    from concourse import bacc, mybir, tile

    FP32 = mybir.dt.float32
    BF16 = mybir.dt.float16  # "BF16" alias now fp16: same perf, 8x finer mantissa
    I16 = mybir.dt.int16
    AX = mybir.AxisListType.X
    OP = mybir.AluOpType
    ACTF = mybir.ActivationFunctionType

    nc = bacc.Bacc(
        None,
        num_devices=NC,
        target_bir_lowering=False,
        debug=False,
        num_swdge_queues=NQ,
    )

    # ---- parameters ----
    HS0DT = BF16
    hs0_d = nc.declare_dram_parameter("hs0", [NP, F_IN], HS0DT, isOutput=False)
    hs0own_d = nc.declare_dram_parameter(
        "hs0own", [128, NWIN, F_IN], HS0DT, isOutput=False
    )
    idx_d = [
        nc.declare_dram_parameter("idxA", [128, TA * 8], I16, isOutput=False),
        nc.declare_dram_parameter("idxB", [128, TB * 8], I16, isOutput=False),
    ]
    stiles_d = [
        nc.declare_dram_parameter("stilesA", [128, TA, 128], BF16, isOutput=False),
        nc.declare_dram_parameter("stilesB", [128, TB, 128], BF16, isOutput=False),
    ]
    dinv_d = nc.declare_dram_parameter("dinvT", [128, SHP], FP32, isOutput=False)
    ident_d = nc.declare_dram_parameter("ident", [128, 128], FP32, isOutput=False)
    W_d = [
        nc.declare_dram_parameter(f"W{j}", list(DIMS[j]), FP32, isOutput=False)
        for j in range(6)
    ]
    gb_d = [
        nc.declare_dram_parameter(f"gb{j}", [128, 2], FP32, isOutput=False)
        for j in range(6)
    ]
    ones_d = nc.declare_dram_parameter("ones", [1, 128], FP32, isOutput=False)
    dinvnb_d = nc.declare_dram_parameter("dinvNB", [128, NWIN], FP32, isOutput=False)
    out_d = nc.declare_dram_parameter("out", [128, SHP], FP32, isOutput=True)

    # ---- internal DRAM: collective bounce buffers ----
    # table for layer j+1 is ag_out[j]; all fp16, 128-wide (64 zero-padded)
    ag_dt = [BF16 for j in range(5)]
    ag_in = [nc.dram_tensor(f"ag_in{j}", [SHP, TBLW], ag_dt[j]) for j in range(5)]
    ag_out = [
        nc.dram_tensor(f"ag_out{j}", [NP, TBLW], ag_dt[j], addr_space="Shared")
        for j in range(5)
    ]
    ar_in = [nc.dram_tensor(f"ar_in{j}", [128, 2], FP32) for j in range(6)]
    ar_out = [
        nc.dram_tensor(f"ar_out{j}", [128, 2], FP32, addr_space="Shared")
        for j in range(6)
    ]

    tbls = [hs0_d] + ag_out

    chunks = [_chunks_of(TA), _chunks_of(TB)]
    idx_col0 = [[], []]
    for h in (0, 1):
        c = 0
        for _, nt in chunks[h]:
            idx_col0[h].append(c)
            c += nt * 8

    with tile.TileContext(nc) as tc:
        with (
            tc.tile_pool(name="res", bufs=1) as res,
            tc.tile_pool(name="msg", bufs=8) as msgp,
            tc.tile_pool(name="sp", bufs=4) as sp,
            tc.tile_pool(name="small", bufs=2) as small,
            tc.tile_pool(name="big", bufs=1) as big,
            tc.tile_pool(name="hx", bufs=2) as hxp,
            tc.tile_pool(name="agg_ps", bufs=2, space="PSUM") as aggp,
            tc.tile_pool(name="y_ps", bufs=2, space="PSUM") as yp,
            tc.tile_pool(name="tr_ps", bufs=2, space="PSUM") as trp,
        ):
            # ---- resident loads ----
            idx_t = [res.tile([128, TA * 8], I16, name="idxA")]
            nc.sync.dma_start(idx_t[0][:], idx_d[0][:])
            idx_t.append(res.tile([128, TB * 8], I16, name="idxB"))
            nc.sync.dma_start(idx_t[1][:], idx_d[1][:])
            ident_f = res.tile([128, 128], FP32, name="identf")
            nc.sync.dma_start(ident_f[:], ident_d[:])
            ident_b = res.tile([128, 128], BF16, name="identb")
            nc.vector.tensor_copy(ident_b[:], ident_f[:])
            ones_t = res.tile([1, 128], FP32, name="ones")
            nc.sync.dma_start(ones_t[:], ones_d[:])
            dinvnb_t = res.tile([128, NWIN], FP32, name="dinvnb")
            nc.sync.dma_start(dinvnb_t[:], dinvnb_d[:])
            W_t = []
            for j in range(6):
                wt = res.tile(list(DIMS[j]), FP32, name=f"W{j}")
                nc.sync.dma_start(wt[:], W_d[j][:])
                W_t.append(wt)
            gb_t = []
            for j in range(6):
                gt = res.tile([128, 2], FP32, name=f"gb{j}")
                nc.sync.dma_start(gt[:], gb_d[j][:])
                gb_t.append(gt)

            # own-shard tile (node-major; source for self-loop transposes)
            hprev = hxp.tile([128, NWIN, F_IN], HS0DT, tag="hx", name="hs0own")
            nc.sync.dma_start(hprev[:], hs0own_d[:])
            hprev_dt = HS0DT

            inv_n = 1.0 / float(N)
            qrr = [0]  # gather queue round-robin counter

            def bn_vec(j, fo, arr_tile):
                """mean/var -> (scale, shift) columns in a [128, 6] tile."""
                vec = small.tile([128, 6], FP32, tag="bnvec", name="vec")
                nc.vector.tensor_scalar(
                    out=vec[0:fo, 0:1], in0=arr_tile[0:fo, 0:1],
                    scalar1=inv_n, scalar2=None, op0=OP.mult,
                )
                nc.vector.tensor_scalar(
                    out=vec[0:fo, 1:2], in0=arr_tile[0:fo, 1:2],
                    scalar1=inv_n, scalar2=None, op0=OP.mult,
                )
                nc.vector.tensor_tensor(
                    vec[0:fo, 2:3], vec[0:fo, 0:1], vec[0:fo, 0:1], op=OP.mult
                )
                nc.vector.tensor_tensor(
                    vec[0:fo, 2:3], vec[0:fo, 1:2], vec[0:fo, 2:3],
                    op=OP.subtract,
                )
                nc.vector.tensor_scalar(
                    out=vec[0:fo, 2:3], in0=vec[0:fo, 2:3],
                    scalar1=float(EPS), scalar2=None, op0=OP.add,
                )
                nc.vector.reciprocal(vec[0:fo, 3:4], vec[0:fo, 2:3])
                nc.scalar.activation(vec[0:fo, 3:4], vec[0:fo, 3:4], ACTF.Sqrt)
                nc.vector.tensor_tensor(
                    vec[0:fo, 4:5], gb_t[j][0:fo, 0:1], vec[0:fo, 3:4],
                    op=OP.mult,
                )
                nc.vector.tensor_tensor(
                    vec[0:fo, 5:6], vec[0:fo, 0:1], vec[0:fo, 4:5], op=OP.mult
                )
                nc.vector.tensor_tensor(
                    vec[0:fo, 5:6], gb_t[j][0:fo, 1:2], vec[0:fo, 5:6],
                    op=OP.subtract,
                )
                return vec

            for j in range(DEBUG_NL):
                fi, fo = DIMS[j]
                tbl = tbls[j]
                MDT = BF16
                ident_in = ident_b if hprev_dt == BF16 else ident_f

                cur_chunk = [-1, -1]
                msg_tiles = [None, None]
                s_tiles = {}

                def ensure_chunk(h, t):
                    k = 0
                    while not (
                        chunks[h][k][0] <= t < chunks[h][k][0] + chunks[h][k][1]
                    ):
                        k += 1
                    if cur_chunk[h] == k:
                        return
                    cur_chunk[h] = k
                    t0c, ntc = chunks[h][k]
                    mt = msgp.tile([128, ntc, TBLW], MDT, tag="msg", name="msg")
                    nc.gpsimd.dma_gather(
                        out_ap=mt[:],
                        in_ap=tbl[h * HALF : (h + 1) * HALF, :],
                        idxs_ap=idx_t[h][
                            :, idx_col0[h][k] : idx_col0[h][k] + ntc * 8
                        ],
                        num_idxs=ntc * 128,
                        num_idxs_reg=ntc * 128,
                        elem_size=TBLW,
                        single_packet=False,
                        queue_num=qrr[0] % NQ,
                    )
                    qrr[0] += 1
                    msg_tiles[h] = (t0c, mt)

                y_sb = big.tile([128, SHP], FP32, tag="ysb", name="ysb")
                sumP = small.tile([128, NSB], FP32, tag="sumP", name="sumP")
                sqP = small.tile([128, NSB], FP32, tag="sqP", name="sqP")
                junk = small.tile([128, 512], FP32, tag="junk", name="junk")

                for sb in range(NSB):
                    nsb = 512 if sb < 12 else 128
                    wlist = list(range(sb * 4, min(sb * 4 + 4, NWIN)))
                    # tile sequence; (w, None, None) = self-loop transpose
                    seq = []
                    for w in wlist:
                        seq.append((w, None, None))
                        for h in (0, 1):
                            for t in range(t0s[w][h], t0s[w][h] + tiles[w][h]):
                                seq.append((w, h, t))
                    agg = aggp.tile([128, 512], FP32, tag="agg", name="agg")
                    for i, (w, h, t) in enumerate(seq):
                        woff = (w % 4) * 128
                        first = i == 0
                        last = i == len(seq) - 1
                        if h is None:
                            # self-loop: agg[f, d] += hprev[d, f], done as a
                            # regular matmul with identity moving operand
                            # (lhsT=hprev) so fp16 input can hit f32 PSUM.
                            nc.tensor.matmul(
                                agg[0:fi, woff : woff + 128],
                                hprev[:, w, 0:fi],
                                ident_in[:],
                                start=first,
                                stop=last,
                            )
                            continue
                        ensure_chunk(h, t)
                        t0c, mt = msg_tiles[h]
                        if (w, h) not in s_tiles:
                            G = tiles[w][h]
                            st = sp.tile([128, G, 128], MDT, tag="S", name="S")
                            nc.sync.dma_start(
                                st[:], stiles_d[h][:, t0s[w][h] : t0s[w][h] + G, :]
                            )
                            s_tiles[(w, h)] = (st, t0s[w][h])
                        st, st_t0 = s_tiles[(w, h)]
                        nc.tensor.matmul(
                            agg[0:fi, woff : woff + 128],
                            mt[:, t - t0c, 0:fi],
                            st[:, t - st_t0, :],
                            start=first,
                            stop=last,
                        )
                    # evict + dinv[dst] scale
                    dv = small.tile([128, 512], FP32, tag="dinv", name="dv")
                    nc.sync.dma_start(
                        dv[:, 0:nsb], dinv_d[:, sb * 512 : sb * 512 + nsb]
                    )
                    rawT = small.tile([128, 512], FP32, tag="rawT", name="rawT")
                    nc.vector.tensor_tensor(
                        rawT[0:fi, 0:nsb],
                        agg[0:fi, 0:nsb],
                        dv[0:fi, 0:nsb],
                        op=OP.mult,
                    )
                    # W matmul (f32)
                    y_ps = yp.tile([128, 512], FP32, tag="yps", name="yps")
                    nc.tensor.matmul(
                        y_ps[0:fo, 0:nsb],
                        W_t[j][:],
                        rawT[0:fi, 0:nsb],
                        start=True,
                        stop=True,
                    )
                    # copy to y_sb + stats over valid columns
                    nv = 512 if sb < 12 else 106
                    c0 = sb * 512
                    nc.scalar.activation(
                        y_sb[0:fo, c0 : c0 + nv],
                        y_ps[0:fo, 0:nv],
                        ACTF.Copy,
                        accum_out=sumP[0:fo, sb : sb + 1],
                    )
                    if sb == 12:
                        nc.scalar.activation(
                            y_sb[0:fo, c0 + 106 : c0 + 128],
                            y_ps[0:fo, 106:128],
                            ACTF.Copy,
                        )
                    nc.scalar.activation(
                        junk[0:fo, 0:nv],
                        y_ps[0:fo, 0:nv],
                        ACTF.Square,
                        accum_out=sqP[0:fo, sb : sb + 1],
                    )

                # ---- kick BN stats all-reduce ----
                stats = small.tile([128, 2], FP32, tag="stats", name="stats")
                nc.vector.memset(stats[:], 0.0)
                nc.vector.reduce_sum(stats[0:fo, 0:1], sumP[0:fo, :], axis=AX)
                nc.vector.reduce_sum(stats[0:fo, 1:2], sqP[0:fo, :], axis=AX)
                nc.sync.dma_start(ar_in[j][:], stats[:])
                nc.gpsimd.collective_compute(
                    "AllReduce",
                    OP.add,
                    replica_groups=[list(range(NC))],
                    ins=[ar_in[j][:]],
                    outs=[ar_out[j][:]],
                )

                if j == DEBUG_NL - 1 and j != 5:
                    pass  # fall through; debug dump happens below
                if j == 5:
                    # final layer: BN in feat-major via ACT, DMA out
                    arr = small.tile([128, 2], FP32, tag="arr", name="arr")
                    nc.sync.dma_start(arr[:], ar_out[j][:])
                    vec = bn_vec(j, fo, arr)
                    for sb in range(NSB):
                        nsb = 512 if sb < 12 else 128
                        c0 = sb * 512
                        nc.scalar.activation(
                            y_sb[0:fo, c0 : c0 + nsb],
                            y_sb[0:fo, c0 : c0 + nsb],
                            ACTF.Identity,
                            bias=vec[0:fo, 5:6],
                            scale=vec[0:fo, 4:5],
                        )
                    nc.sync.dma_start(out_d[:], y_sb[:])
                    continue

                # ---- transpose raw y to node-major (overlaps the AR) ----
                hnext = hxp.tile([128, NWIN, TBLW], ag_dt[j], tag="hx", name="hnext")
                if fo < TBLW:
                    nc.vector.memset(hnext[:, :, fo:TBLW], 0.0)
                for b0 in range(0, NWIN, 4):
                    nb = min(4, NWIN - b0)
                    tr4 = trp.tile([128, 512], FP32, tag="tr", name="tr4")
                    for bi in range(nb):
                        b = b0 + bi
                        nc.tensor.matmul(
                            tr4[0:128, bi * fo : bi * fo + fo],
                            y_sb[0:fo, b * 128 : (b + 1) * 128],
                            ident_f[0:fo, 0:fo],
                            is_transpose=True,
                            start=True,
                            stop=True,
                        )
                    nc.vector.tensor_copy(
                        hnext[:, b0 : b0 + nb, 0:fo],
                        tr4[:, 0 : nb * fo].rearrange("p (b f) -> p b f", f=fo),
                    )

                # ---- AR result -> scale/shift -> rank-1 broadcast tiles ----
                arr = small.tile([128, 2], FP32, tag="arr", name="arr")
                nc.sync.dma_start(arr[:], ar_out[j][:])
                vec = bn_vec(j, fo, arr)
                # transpose scale/shift columns to rows, then rank-1 bcast
                scale_rep = small.tile(
                    [128, 128], ag_dt[j], tag="srep", name="scale_rep"
                )
                shift_rep = small.tile(
                    [128, 128], ag_dt[j], tag="hrep", name="shift_rep"
                )
                for col, rep in ((4, scale_rep), (5, shift_rep)):
                    vt_ps = trp.tile([128, 128], FP32, tag="tr", name="vtps")
                    nc.tensor.matmul(
                        vt_ps[0:1, 0:fo],
                        vec[0:fo, col : col + 1],
                        ident_f[0:fo, 0:fo],
                        is_transpose=True,
                        start=True,
                        stop=True,
                    )
                    vrow = small.tile([1, 128], FP32, tag="vrow", name="vrow")
                    nc.vector.tensor_copy(vrow[:, 0:fo], vt_ps[0:1, 0:fo])
                    rep_ps = trp.tile([128, 128], FP32, tag="tr", name="repps")
                    nc.tensor.matmul(
                        rep_ps[:, 0:fo], ones_t[:], vrow[0:1, 0:fo],
                        start=True, stop=True,
                    )
                    nc.vector.tensor_copy(rep[:, 0:fo], rep_ps[:, 0:fo])

                if DEBUG_REPS and j == DEBUG_NL - 1:
                    nc.sync.dma_start(out_d[:, 0:128], scale_rep[:])
                    nc.sync.dma_start(out_d[:, 128:256], shift_rep[:])
                    nc.sync.dma_start(out_d[:, 256:262], vec[:, 0:6])
                    continue
                # ---- BN apply (+ReLU) in node-major, in place ----
                for b in range(NWIN if not (DEBUG_PREBN and j == DEBUG_NL - 1) else 0):
                    blk = hnext[:, b, 0:fo]
                    nc.vector.tensor_tensor(
                        blk, blk, scale_rep[:, 0:fo], op=OP.mult
                    )
                    nc.vector.tensor_tensor(
                        blk, blk, shift_rep[:, 0:fo], op=OP.add
                    )
                    if RELU[j]:
                        nc.vector.tensor_scalar(
                            out=blk, in0=blk,
                            scalar1=dinvnb_t[:, b : b + 1], scalar2=0.0,
                            op0=OP.mult, op1=OP.max,
                        )
                    else:
                        nc.vector.tensor_scalar(
                            out=blk, in0=blk,
                            scalar1=dinvnb_t[:, b : b + 1], scalar2=None,
                            op0=OP.mult,
                        )

                if j == DEBUG_NL - 1:
                    nc.sync.dma_start(
                        out_d[:, 0 : NWIN * fo].rearrange(
                            "p (b f) -> p b f", f=fo
                        ),
                        hnext[:, :, 0:fo],
                    )
                    continue
                nc.sync.dma_start(
                    ag_in[j][:].rearrange("(b p) f -> p b f", p=128), hnext[:]
                )
                nc.gpsimd.collective_compute(
                    "AllGather",
                    OP.bypass,
                    replica_groups=[list(range(NC))],
                    ins=[ag_in[j][:]],
                    outs=[ag_out[j][:]],
                )
                hprev = hnext
                hprev_dt = ag_dt[j]

    nc.compile()
    return nc


def kernel(x, edge_index, **params):
    global LAST_RESULT
    import ml_dtypes

    from concourse.bass_utils import run_bass_kernel_spmd

    x = np.asarray(x, np.float32)
    edge_index = np.asarray(edge_index, np.int64)
    src_all = edge_index[0]
    dst_all = edge_index[1]

    deg = (np.bincount(dst_all, minlength=N) + 1.0).astype(np.float32)
    dinv = (1.0 / np.sqrt(deg)).astype(np.float32)

    hs0 = np.zeros((NP, F_IN), np.float32)
    xs = x * dinv[:, None]
    for c in range(NC):
        hs0[c * SHP : c * SHP + SH] = xs[c * SH : (c + 1) * SH]
    hs0_bf = hs0.astype(np.float16)

    remap = (src_all // SH) * SHP + (src_all % SH)

    eds = []
    for c in range(NC):
        m = (dst_all >= c * SH) & (dst_all < (c + 1) * SH)
        dstl = dst_all[m] - c * SH
        srcr = remap[m]
        eds.append(_prep_edges(srcr, dstl))

    tiles = [[0, 0] for _ in range(NWIN)]
    for w in range(NWIN):
        for h in (0, 1):
            mx = max(len(eds[c][w][h][0]) for c in range(NC))
            tiles[w][h] = -(-mx // 128) if mx else 0
    t0s = [[0, 0] for _ in range(NWIN)]
    ta = tb = 0
    for w in range(NWIN):
        t0s[w][0] = ta
        ta += tiles[w][0]
        t0s[w][1] = tb
        tb += tiles[w][1]
    TA, TB = ta, tb

    chunksA = _chunks_of(TA)
    chunksB = _chunks_of(TB)

    def _build_stiles(drel, T):
        """One-hot S tiles, partition-major [128, T, 128] fp16 (pad=-1)."""
        s = np.zeros((T * 128, 128), np.float16)
        rel = drel.astype(np.int64)
        valid = np.nonzero(rel >= 0)[0]
        s[valid, rel[valid]] = 1.0
        return np.ascontiguousarray(s.reshape(T, 128, 128).transpose(1, 0, 2))

    in_maps = []
    for c in range(NC):
        (gA, dA), (gB, dB) = _build_core_tables(eds[c], tiles)
        dinvT = np.zeros(SHP, np.float32)
        dinvT[:SH] = dinv[c * SH : (c + 1) * SH]
        # own shard in [128, NWIN, F] node-major layout: [p, b, :] = node b*128+p
        own = hs0_bf[c * SHP : (c + 1) * SHP].reshape(NWIN, 128, F_IN)
        im = {
            "hs0": hs0_bf,
            "hs0own": np.ascontiguousarray(own.transpose(1, 0, 2)),
            "idxA": _wrap_idx(gA, chunksA),
            "idxB": _wrap_idx(gB, chunksB),
            "stilesA": _build_stiles(dA, TA),
            "stilesB": _build_stiles(dB, TB),
            "dinvT": np.broadcast_to(dinvT, (128, SHP)).copy(),
            "ident": np.eye(128, dtype=np.float32),
            "ones": np.ones((1, 128), np.float32),
            "dinvNB": np.ascontiguousarray(
                dinvT.reshape(NWIN, 128).T
            ),
        }
        for j in range(6):
            im[f"W{j}"] = np.asarray(params[f"W{j}"], np.float32)
            gb = np.zeros((128, 2), np.float32)
            fo = DIMS[j][1]
            gb[:fo, 0] = np.asarray(params[f"g{j}"], np.float32)
            gb[:fo, 1] = np.asarray(params[f"be{j}"], np.float32)
            im[f"gb{j}"] = gb
        in_maps.append(im)

    nc = _build_program(tiles, t0s, TA, TB)
    res = run_bass_kernel_spmd(
        nc,
        in_maps,
        core_ids=list(range(NC)),
        trace=TRACE,
        **TRACE_KW,
    )
    LAST_RESULT = res

    out = np.empty((N, F_IN), np.float32)
    for c in range(NC):
        out[c * SH : (c + 1) * SH] = res.results[c]["out"].T[:SH]
    return out



# revision 26
# speedup vs baseline: 1.1002x; 1.0034x over previous
"""Trainium2 Bass kernel for nn_AutoEncoder (6-layer GCN autoencoder).

Strategy (8 NeuronCores, SPMD):
  - Destination nodes sharded across cores (6250/core, padded to 6272).
  - Node features kept pre-scaled by deg^-1/2 ("hs") and replicated on every
    core in a padded [8*6272, F] layout (per-layer AllGather, bf16 except the
    64-wide bottleneck layer which must stay f32 for the 256B-row DMA-gather
    constraint).
  - Per layer: dma_gather of hs[src] for this core's edges (edge list sorted
    by local dst, split by int16-index halves), segment-sum via one-hot
    matmuls accumulated in PSUM (128-dst windows); self-loop contributions
    enter the same PSUM banks as PE transposes of the resident own-shard
    tile. Aggregate is scaled by deg^-1/2[dst] on eviction, W matmul in f32,
    then raw y is PE-transposed to node-major while the BatchNorm (sum,
    sumsq) AllReduce is in flight; BN + ReLU + deg^-1/2 rescale are applied
    post-AllReduce in node-major via rank-1 broadcast tiles.
  - The GCN bias b is skipped (training-mode BatchNorm makes any per-feature
    constant shift a no-op).
"""

import sys

sys.path.insert(0, "/opt/trn_rl_repo")

import numpy as np

N = 50000
E = 800000
F_IN = 128
EPS = 1e-5
NC = 8
SH = 6250  # real dst nodes per core
SHP = 6272  # padded (49 * 128)
NP = NC * SHP  # 50176 rows in the padded replicated node table
HALF = NP // 2  # 25088 (< int16 max) rows per gather table half
WIN = 128  # dst window = psum column band
NWIN = SHP // WIN  # 49
NSB = 13  # psum superblocks: 12 x 512 + 1 x 128
CHUNK = 32  # gather chunk size in K-tiles
NQ = 4  # SWDGE queues (round-robin; each runs on its own Q7 core pair)
DIMS = [(128, 128), (128, 128), (128, 64), (64, 128), (128, 128), (128, 128)]
RELU = [True, True, False, True, True, False]
TBLW = 128  # table row width (64-wide bottleneck zero-padded to 128, fp16)

import os as _os
DEBUG_NL = int(_os.environ.get("DEBUG_NL", "6"))  # layers to run (debug)
DEBUG_PREBN = int(_os.environ.get("DEBUG_PREBN", "0"))
DEBUG_REPS = int(_os.environ.get("DEBUG_REPS", "0"))
TRACE = False  # set by test.py for profiling runs
TRACE_KW = {}
LAST_RESULT = None  # BassKernelResults of the last run (for test.py)


def _prep_edges(src_remap, dstl):
    """Per-core edge prep: sort by dst; per-(window, half) edge lists."""
    order = np.argsort(dstl, kind="stable")
    dstl = dstl[order]
    srcr = src_remap[order]
    half = (srcr >= HALF).astype(np.int64)
    w = dstl // WIN
    rel = dstl - w * WIN
    ed = [[None, None] for _ in range(NWIN)]
    for wi in range(NWIN):
        m = w == wi
        for h in (0, 1):
            mh = m & (half == h)
            ed[wi][h] = (srcr[mh] - h * HALF, rel[mh])
    return ed


def _build_core_tables(ed, tiles):
    """Pack per-core edge lists into padded tile streams (per half)."""
    out = []
    for h in (0, 1):
        T = sum(tiles[w][h] for w in range(NWIN))
        gidx = np.zeros(T * 128, np.int16)
        drel = np.full(T * 128, -1.0, np.float32)
        t = 0
        for w in range(NWIN):
            g, r = ed[w][h]
            nt = tiles[w][h]
            assert len(g) <= nt * 128
            base = t * 128
            gidx[base : base + len(g)] = g.astype(np.int16)
            drel[base : base + len(g)] = r.astype(np.float32)
            t += nt
        out.append((gidx, drel))
    return out


def _wrap_idx(gidx, chunks):
    """int16 indices -> [128, total/16] wrapped per chunk, tiled 8x."""
    total_cols = len(gidx) // 16
    arr = np.zeros((16, total_cols), np.int16)
    col = 0
    for t0, nt in chunks:
        cidx = gidx[t0 * 128 : (t0 + nt) * 128]
        ncol = len(cidx) // 16
        arr[:, col : col + ncol] = cidx.reshape(ncol, 16).T
        col += ncol
    assert col == total_cols
    return np.tile(arr, (8, 1)).copy()


def _chunks_of(T):
    out = []
    t = 0
    while t < T:
        nt = min(CHUNK, T - t)
        out.append((t, nt))
        t += nt
    return out


def _build_program(tiles, t0s, TA, TB):
    from concourse import bacc, mybir, tile

    FP32 = mybir.dt.float32
    BF16 = mybir.dt.float16  # "BF16" alias now fp16: same perf, 8x finer mantissa
    I16 = mybir.dt.int16
    AX = mybir.AxisListType.X
    OP = mybir.AluOpType
    ACTF = mybir.ActivationFunctionType

    nc = bacc.Bacc(
        None,
        num_devices=NC,
        target_bir_lowering=False,
        debug=False,
        num_swdge_queues=NQ,
    )

    # ---- parameters ----
    HS0DT = BF16
    hs0_d = nc.declare_dram_parameter("hs0", [NP, F_IN], HS0DT, isOutput=False)
    hs0own_d = nc.declare_dram_parameter(
        "hs0own", [128, NWIN, F_IN], HS0DT, isOutput=False
    )
    idx_d = [
        nc.declare_dram_parameter("idxA", [128, TA * 8], I16, isOutput=False),
        nc.declare_dram_parameter("idxB", [128, TB * 8], I16, isOutput=False),
    ]
    stiles_d = [
        nc.declare_dram_parameter("stilesA", [128, TA, 128], BF16, isOutput=False),
        nc.declare_dram_parameter("stilesB", [128, TB, 128], BF16, isOutput=False),
    ]
    dinv_d = nc.declare_dram_parameter("dinvT", [128, SHP], FP32, isOutput=False)
    ident_d = nc.declare_dram_parameter("ident", [128, 128], FP32, isOutput=False)
    W_d = [
        nc.declare_dram_parameter(f"W{j}", list(DIMS[j]), FP32, isOutput=False)
        for j in range(6)
    ]
    gb_d = [
        nc.declare_dram_parameter(f"gb{j}", [128, 2], FP32, isOutput=False)
        for j in range(6)
    ]
    ones_d = nc.declare_dram_parameter("ones", [1, 128], FP32, isOutput=False)
    dinvnb_d = nc.declare_dram_parameter("dinvNB", [128, NWIN], FP32, isOutput=False)
    out_d = nc.declare_dram_parameter("out", [128, SHP], FP32, isOutput=True)

    # ---- internal DRAM: collective bounce buffers ----
    # table for layer j+1 is ag_out[j]; all fp16, 128-wide (64 zero-padded)
    ag_dt = [BF16 for j in range(5)]
    ag_in = [nc.dram_tensor(f"ag_in{j}", [SHP, TBLW], ag_dt[j]) for j in range(5)]
    ag_out = [
        nc.dram_tensor(f"ag_out{j}", [NP, TBLW], ag_dt[j], addr_space="Shared")
        for j in range(5)
    ]
    ar_in = [nc.dram_tensor(f"ar_in{j}", [128, 2], FP32) for j in range(6)]
    ar_out = [
        nc.dram_tensor(f"ar_out{j}", [128, 2], FP32, addr_space="Shared")
        for j in range(6)
    ]

    tbls = [hs0_d] + ag_out

    chunks = [_chunks_of(TA), _chunks_of(TB)]
    idx_col0 = [[], []]
    for h in (0, 1):
        c = 0
        for _, nt in chunks[h]:
            idx_col0[h].append(c)
            c += nt * 8

    with tile.TileContext(nc) as tc:
        with (
            tc.tile_pool(name="res", bufs=1) as res,
            tc.tile_pool(name="msg", bufs=8) as msgp,
            tc.tile_pool(name="sp", bufs=4) as sp,
            tc.tile_pool(name="small", bufs=2) as small,
            tc.tile_pool(name="big", bufs=1) as big,
            tc.tile_pool(name="hx", bufs=2) as hxp,
            tc.tile_pool(name="agg_ps", bufs=2, space="PSUM") as aggp,
            tc.tile_pool(name="y_ps", bufs=2, space="PSUM") as yp,
            tc.tile_pool(name="tr_ps", bufs=2, space="PSUM") as trp,
        ):
            # ---- resident loads ----
            idx_t = [res.tile([128, TA * 8], I16, name="idxA")]
            nc.sync.dma_start(idx_t[0][:], idx_d[0][:])
            idx_t.append(res.tile([128, TB * 8], I16, name="idxB"))
            nc.sync.dma_start(idx_t[1][:], idx_d[1][:])
            ident_f = res.tile([128, 128], FP32, name="identf")
            nc.sync.dma_start(ident_f[:], ident_d[:])
            ident_b = res.tile([128, 128], BF16, name="identb")
            nc.vector.tensor_copy(ident_b[:], ident_f[:])
            ones_t = res.tile([1, 128], FP32, name="ones")
            nc.sync.dma_start(ones_t[:], ones_d[:])
            dinvnb_t = res.tile([128, NWIN], FP32, name="dinvnb")
            nc.sync.dma_start(dinvnb_t[:], dinvnb_d[:])
            W_t = []
            for j in range(6):
                wt = res.tile(list(DIMS[j]), FP32, name=f"W{j}")
                nc.sync.dma_start(wt[:], W_d[j][:])
                W_t.append(wt)
            gb_t = []
            for j in range(6):
                gt = res.tile([128, 2], FP32, name=f"gb{j}")
                nc.sync.dma_start(gt[:], gb_d[j][:])
                gb_t.append(gt)

            # own-shard tile (node-major; source for self-loop transposes)
            hprev = hxp.tile([128, NWIN, F_IN], HS0DT, tag="hx", name="hs0own")
            nc.sync.dma_start(hprev[:], hs0own_d[:])
            hprev_dt = HS0DT

            inv_n = 1.0 / float(N)
            qrr = [0]  # gather queue round-robin counter

            def bn_vec(j, fo, arr_tile):
                """mean/var -> (scale, shift) columns in a [128, 6] tile."""
                vec = small.tile([128, 6], FP32, tag="bnvec", name="vec")
                nc.vector.tensor_scalar(
                    out=vec[0:fo, 0:1], in0=arr_tile[0:fo, 0:1],
                    scalar1=inv_n, scalar2=None, op0=OP.mult,
                )
                nc.vector.tensor_scalar(
                    out=vec[0:fo, 1:2], in0=arr_tile[0:fo, 1:2],
                    scalar1=inv_n, scalar2=None, op0=OP.mult,
                )
                nc.vector.tensor_tensor(
                    vec[0:fo, 2:3], vec[0:fo, 0:1], vec[0:fo, 0:1], op=OP.mult
                )
                nc.vector.tensor_tensor(
                    vec[0:fo, 2:3], vec[0:fo, 1:2], vec[0:fo, 2:3],
                    op=OP.subtract,
                )
                nc.vector.tensor_scalar(
                    out=vec[0:fo, 2:3], in0=vec[0:fo, 2:3],
                    scalar1=float(EPS), scalar2=None, op0=OP.add,
                )
                nc.vector.reciprocal(vec[0:fo, 3:4], vec[0:fo, 2:3])
                nc.scalar.activation(vec[0:fo, 3:4], vec[0:fo, 3:4], ACTF.Sqrt)
                nc.vector.tensor_tensor(
                    vec[0:fo, 4:5], gb_t[j][0:fo, 0:1], vec[0:fo, 3:4],
                    op=OP.mult,
                )
                nc.vector.tensor_tensor(
                    vec[0:fo, 5:6], vec[0:fo, 0:1], vec[0:fo, 4:5], op=OP.mult
                )
                nc.vector.tensor_tensor(
                    vec[0:fo, 5:6], gb_t[j][0:fo, 1:2], vec[0:fo, 5:6],
                    op=OP.subtract,
                )
                return vec

            for j in range(DEBUG_NL):
                fi, fo = DIMS[j]
                tbl = tbls[j]
                MDT = BF16
                ident_in = ident_b if hprev_dt == BF16 else ident_f

                cur_chunk = [-1, -1]
                msg_tiles = [None, None]

                def ensure_chunk(h, t):
                    k = 0
                    while not (
                        chunks[h][k][0] <= t < chunks[h][k][0] + chunks[h][k][1]
                    ):
                        k += 1
                    if cur_chunk[h] == k:
                        return
                    cur_chunk[h] = k
                    t0c, ntc = chunks[h][k]
                    mt = msgp.tile([128, ntc, TBLW], MDT, tag="msg", name="msg")
                    nc.gpsimd.dma_gather(
                        out_ap=mt[:],
                        in_ap=tbl[h * HALF : (h + 1) * HALF, :],
                        idxs_ap=idx_t[h][
                            :, idx_col0[h][k] : idx_col0[h][k] + ntc * 8
                        ],
                        num_idxs=ntc * 128,
                        num_idxs_reg=ntc * 128,
                        elem_size=TBLW,
                        single_packet=False,
                        queue_num=qrr[0] % NQ,
                    )
                    qrr[0] += 1
                    msg_tiles[h] = (t0c, mt)

                y_sb = big.tile([128, SHP], FP32, tag="ysb", name="ysb")
                sumP = small.tile([128, NSB], FP32, tag="sumP", name="sumP")
                sqP = small.tile([128, NSB], FP32, tag="sqP", name="sqP")
                junk = small.tile([128, 512], FP32, tag="junk", name="junk")

                for sb in range(NSB):
                    nsb = 512 if sb < 12 else 128
                    wlist = list(range(sb * 4, min(sb * 4 + 4, NWIN)))
                    # tile sequence; (w, None, None) = self-loop transpose
                    seq = []
                    for w in wlist:
                        seq.append((w, None, None))
                        for h in (0, 1):
                            for t in range(t0s[w][h], t0s[w][h] + tiles[w][h]):
                                seq.append((w, h, t))
                    # batched S load: one DMA per half covering this sb's windows
                    st_h = [None, None]
                    st_base = [0, 0]
                    for h in (0, 1):
                        tb0 = t0s[wlist[0]][h]
                        tb1 = t0s[wlist[-1]][h] + tiles[wlist[-1]][h]
                        st_base[h] = tb0
                        if tb1 > tb0:
                            st = sp.tile(
                                [128, tb1 - tb0, 128], MDT, tag="S", name="S"
                            )
                            nc.sync.dma_start(st[:], stiles_d[h][:, tb0:tb1, :])
                            st_h[h] = st
                    agg = aggp.tile([128, 512], FP32, tag="agg", name="agg")
                    for i, (w, h, t) in enumerate(seq):
                        woff = (w % 4) * 128
                        first = i == 0
                        last = i == len(seq) - 1
                        if h is None:
                            # self-loop: agg[f, d] += hprev[d, f], done as a
                            # regular matmul with identity moving operand
                            # (lhsT=hprev) so fp16 input can hit f32 PSUM.
                            nc.tensor.matmul(
                                agg[0:fi, woff : woff + 128],
                                hprev[:, w, 0:fi],
                                ident_in[:],
                                start=first,
                                stop=last,
                            )
                            continue
                        ensure_chunk(h, t)
                        t0c, mt = msg_tiles[h]
                        nc.tensor.matmul(
                            agg[0:fi, woff : woff + 128],
                            mt[:, t - t0c, 0:fi],
                            st_h[h][:, t - st_base[h], :],
                            start=first,
                            stop=last,
                        )
                    # evict + dinv[dst] scale
                    dv = small.tile([128, 512], FP32, tag="dinv", name="dv")
                    nc.sync.dma_start(
                        dv[:, 0:nsb], dinv_d[:, sb * 512 : sb * 512 + nsb]
                    )
                    rawT = small.tile([128, 512], FP32, tag="rawT", name="rawT")
                    nc.vector.tensor_tensor(
                        rawT[0:fi, 0:nsb],
                        agg[0:fi, 0:nsb],
                        dv[0:fi, 0:nsb],
                        op=OP.mult,
                    )
                    # W matmul (f32)
                    y_ps = yp.tile([128, 512], FP32, tag="yps", name="yps")
                    nc.tensor.matmul(
                        y_ps[0:fo, 0:nsb],
                        W_t[j][:],
                        rawT[0:fi, 0:nsb],
                        start=True,
                        stop=True,
                    )
                    # copy to y_sb + stats over valid columns
                    nv = 512 if sb < 12 else 106
                    c0 = sb * 512
                    nc.scalar.activation(
                        y_sb[0:fo, c0 : c0 + nv],
                        y_ps[0:fo, 0:nv],
                        ACTF.Copy,
                        accum_out=sumP[0:fo, sb : sb + 1],
                    )
                    if sb == 12:
                        nc.scalar.activation(
                            y_sb[0:fo, c0 + 106 : c0 + 128],
                            y_ps[0:fo, 106:128],
                            ACTF.Copy,
                        )
                    nc.scalar.activation(
                        junk[0:fo, 0:nv],
                        y_ps[0:fo, 0:nv],
                        ACTF.Square,
                        accum_out=sqP[0:fo, sb : sb + 1],
                    )

                # ---- kick BN stats all-reduce ----
                stats = small.tile([128, 2], FP32, tag="stats", name="stats")
                nc.vector.memset(stats[:], 0.0)
                nc.vector.reduce_sum(stats[0:fo, 0:1], sumP[0:fo, :], axis=AX)
                nc.vector.reduce_sum(stats[0:fo, 1:2], sqP[0:fo, :], axis=AX)
                nc.sync.dma_start(ar_in[j][:], stats[:])
                nc.gpsimd.collective_compute(
                    "AllReduce",
                    OP.add,
                    replica_groups=[list(range(NC))],
                    ins=[ar_in[j][:]],
                    outs=[ar_out[j][:]],
                )

                if j == DEBUG_NL - 1 and j != 5:
                    pass  # fall through; debug dump happens below
                if j == 5:
                    # final layer: BN in feat-major via ACT, DMA out
                    arr = small.tile([128, 2], FP32, tag="arr", name="arr")
                    nc.sync.dma_start(arr[:], ar_out[j][:])
                    vec = bn_vec(j, fo, arr)
                    for sb in range(NSB):
                        nsb = 512 if sb < 12 else 128
                        c0 = sb * 512
                        nc.scalar.activation(
                            y_sb[0:fo, c0 : c0 + nsb],
                            y_sb[0:fo, c0 : c0 + nsb],
                            ACTF.Identity,
                            bias=vec[0:fo, 5:6],
                            scale=vec[0:fo, 4:5],
                        )
                    nc.sync.dma_start(out_d[:], y_sb[:])
                    continue

                # ---- transpose raw y to node-major (overlaps the AR) ----
                hnext = hxp.tile([128, NWIN, TBLW], ag_dt[j], tag="hx", name="hnext")
                if fo < TBLW:
                    nc.vector.memset(hnext[:, :, fo:TBLW], 0.0)
                for b0 in range(0, NWIN, 4):
                    nb = min(4, NWIN - b0)
                    tr4 = trp.tile([128, 512], FP32, tag="tr", name="tr4")
                    for bi in range(nb):
                        b = b0 + bi
                        nc.tensor.matmul(
                            tr4[0:128, bi * fo : bi * fo + fo],
                            y_sb[0:fo, b * 128 : (b + 1) * 128],
                            ident_f[0:fo, 0:fo],
                            is_transpose=True,
                            start=True,
                            stop=True,
                        )
                    nc.vector.tensor_copy(
                        hnext[:, b0 : b0 + nb, 0:fo],
                        tr4[:, 0 : nb * fo].rearrange("p (b f) -> p b f", f=fo),
                    )

                # ---- AR result -> scale/shift -> rank-1 broadcast tiles ----
                arr = small.tile([128, 2], FP32, tag="arr", name="arr")
                nc.sync.dma_start(arr[:], ar_out[j][:])
                vec = bn_vec(j, fo, arr)
                # transpose scale/shift columns to rows, then rank-1 bcast
                scale_rep = small.tile(
                    [128, 128], ag_dt[j], tag="srep", name="scale_rep"
                )
                shift_rep = small.tile(
                    [128, 128], ag_dt[j], tag="hrep", name="shift_rep"
                )
                for col, rep in ((4, scale_rep), (5, shift_rep)):
                    vt_ps = trp.tile([128, 128], FP32, tag="tr", name="vtps")
                    nc.tensor.matmul(
                        vt_ps[0:1, 0:fo],
                        vec[0:fo, col : col + 1],
                        ident_f[0:fo, 0:fo],
                        is_transpose=True,
                        start=True,
                        stop=True,
                    )
                    vrow = small.tile([1, 128], FP32, tag="vrow", name="vrow")
                    nc.vector.tensor_copy(vrow[:, 0:fo], vt_ps[0:1, 0:fo])
                    rep_ps = trp.tile([128, 128], FP32, tag="tr", name="repps")
                    nc.tensor.matmul(
                        rep_ps[:, 0:fo], ones_t[:], vrow[0:1, 0:fo],
                        start=True, stop=True,
                    )
                    nc.vector.tensor_copy(rep[:, 0:fo], rep_ps[:, 0:fo])

                if DEBUG_REPS and j == DEBUG_NL - 1:
                    nc.sync.dma_start(out_d[:, 0:128], scale_rep[:])
                    nc.sync.dma_start(out_d[:, 128:256], shift_rep[:])
                    nc.sync.dma_start(out_d[:, 256:262], vec[:, 0:6])
                    continue
                # ---- BN apply (+ReLU) in node-major, in place ----
                for b in range(NWIN if not (DEBUG_PREBN and j == DEBUG_NL - 1) else 0):
                    blk = hnext[:, b, 0:fo]
                    nc.vector.tensor_tensor(
                        blk, blk, scale_rep[:, 0:fo], op=OP.mult
                    )
                    nc.vector.tensor_tensor(
                        blk, blk, shift_rep[:, 0:fo], op=OP.add
                    )
                    if RELU[j]:
                        nc.vector.tensor_scalar(
                            out=blk, in0=blk,
                            scalar1=dinvnb_t[:, b : b + 1], scalar2=0.0,
                            op0=OP.mult, op1=OP.max,
                        )
                    else:
                        nc.vector.tensor_scalar(
                            out=blk, in0=blk,
                            scalar1=dinvnb_t[:, b : b + 1], scalar2=None,
                            op0=OP.mult,
                        )

                if j == DEBUG_NL - 1:
                    nc.sync.dma_start(
                        out_d[:, 0 : NWIN * fo].rearrange(
                            "p (b f) -> p b f", f=fo
                        ),
                        hnext[:, :, 0:fo],
                    )
                    continue
                nc.sync.dma_start(
                    ag_in[j][:].rearrange("(b p) f -> p b f", p=128), hnext[:]
                )
                nc.gpsimd.collective_compute(
                    "AllGather",
                    OP.bypass,
                    replica_groups=[list(range(NC))],
                    ins=[ag_in[j][:]],
                    outs=[ag_out[j][:]],
                )
                hprev = hnext
                hprev_dt = ag_dt[j]

    nc.compile()
    return nc


def kernel(x, edge_index, **params):
    global LAST_RESULT
    import ml_dtypes

    from concourse.bass_utils import run_bass_kernel_spmd

    x = np.asarray(x, np.float32)
    edge_index = np.asarray(edge_index, np.int64)
    src_all = edge_index[0]
    dst_all = edge_index[1]

    deg = (np.bincount(dst_all, minlength=N) + 1.0).astype(np.float32)
    dinv = (1.0 / np.sqrt(deg)).astype(np.float32)

    hs0 = np.zeros((NP, F_IN), np.float32)
    xs = x * dinv[:, None]
    for c in range(NC):
        hs0[c * SHP : c * SHP + SH] = xs[c * SH : (c + 1) * SH]
    hs0_bf = hs0.astype(np.float16)

    remap = (src_all // SH) * SHP + (src_all % SH)

    eds = []
    for c in range(NC):
        m = (dst_all >= c * SH) & (dst_all < (c + 1) * SH)
        dstl = dst_all[m] - c * SH
        srcr = remap[m]
        eds.append(_prep_edges(srcr, dstl))

    tiles = [[0, 0] for _ in range(NWIN)]
    for w in range(NWIN):
        for h in (0, 1):
            mx = max(len(eds[c][w][h][0]) for c in range(NC))
            tiles[w][h] = -(-mx // 128) if mx else 0
    t0s = [[0, 0] for _ in range(NWIN)]
    ta = tb = 0
    for w in range(NWIN):
        t0s[w][0] = ta
        ta += tiles[w][0]
        t0s[w][1] = tb
        tb += tiles[w][1]
    TA, TB = ta, tb

    chunksA = _chunks_of(TA)
    chunksB = _chunks_of(TB)

    def _build_stiles(drel, T):
        """One-hot S tiles, partition-major [128, T, 128] fp16 (pad=-1)."""
        s = np.zeros((T * 128, 128), np.float16)
        rel = drel.astype(np.int64)
        valid = np.nonzero(rel >= 0)[0]
        s[valid, rel[valid]] = 1.0
        return np.ascontiguousarray(s.reshape(T, 128, 128).transpose(1, 0, 2))

    in_maps = []
    for c in range(NC):
        (gA, dA), (gB, dB) = _build_core_tables(eds[c], tiles)
        dinvT = np.zeros(SHP, np.float32)
        dinvT[:SH] = dinv[c * SH : (c + 1) * SH]
        # own shard in [128, NWIN, F] node-major layout: [p, b, :] = node b*128+p
        own = hs0_bf[c * SHP : (c + 1) * SHP].reshape(NWIN, 128, F_IN)
        im = {
            "hs0": hs0_bf,
            "hs0own": np.ascontiguousarray(own.transpose(1, 0, 2)),
            "idxA": _wrap_idx(gA, chunksA),
            "idxB": _wrap_idx(gB, chunksB),
            "stilesA": _build_stiles(dA, TA),
            "stilesB": _build_stiles(dB, TB),
            "dinvT": np.broadcast_to(dinvT, (128, SHP)).copy(),
            "ident": np.eye(128, dtype=np.float32),
            "ones": np.ones((1, 128), np.float32),
            "dinvNB": np.ascontiguousarray(
                dinvT.reshape(NWIN, 128).T
            ),
        }
        for j in range(6):
            im[f"W{j}"] = np.asarray(params[f"W{j}"], np.float32)
            gb = np.zeros((128, 2), np.float32)
            fo = DIMS[j][1]
            gb[:fo, 0] = np.asarray(params[f"g{j}"], np.float32)
            gb[:fo, 1] = np.asarray(params[f"be{j}"], np.float32)
            im[f"gb{j}"] = gb
        in_maps.append(im)

    nc = _build_program(tiles, t0s, TA, TB)
    res = run_bass_kernel_spmd(
        nc,
        in_maps,
        core_ids=list(range(NC)),
        trace=TRACE,
        **TRACE_KW,
    )
    LAST_RESULT = res

    out = np.empty((N, F_IN), np.float32)
    for c in range(NC):
        out[c * SH : (c + 1) * SH] = res.results[c]["out"].T[:SH]
    return out



# revision 36
# speedup vs baseline: 1.3035x; 1.1848x over previous
"""Trainium2 Bass kernel for nn_AutoEncoder (6-layer GCN autoencoder).

Strategy (8 NeuronCores, SPMD):
  - Destination nodes sharded across cores (6250/core, padded to 6272).
  - Node features kept pre-scaled by deg^-1/2 ("hs") and replicated on every
    core in a padded [8*6272, F] layout (per-layer AllGather, bf16 except the
    64-wide bottleneck layer which must stay f32 for the 256B-row DMA-gather
    constraint).
  - Per layer: dma_gather of hs[src] for this core's edges (edge list sorted
    by local dst, split by int16-index halves), segment-sum via one-hot
    matmuls accumulated in PSUM (128-dst windows); self-loop contributions
    enter the same PSUM banks as PE transposes of the resident own-shard
    tile. Aggregate is scaled by deg^-1/2[dst] on eviction, W matmul in f32,
    then raw y is PE-transposed to node-major while the BatchNorm (sum,
    sumsq) AllReduce is in flight; BN + ReLU + deg^-1/2 rescale are applied
    post-AllReduce in node-major via rank-1 broadcast tiles.
  - The GCN bias b is skipped (training-mode BatchNorm makes any per-feature
    constant shift a no-op).
"""

import sys

sys.path.insert(0, "/opt/trn_rl_repo")

import numpy as np

N = 50000
E = 800000
F_IN = 128
EPS = 1e-5
NC = 8
SH = 6250  # real dst nodes per core
SHP = 6272  # padded (49 * 128)
NP = NC * SHP  # 50176 rows in the padded replicated node table
HALF = NP // 2  # 25088 (< int16 max) rows per gather table half
WIN = 128  # dst window = psum column band
NWIN = SHP // WIN  # 49
NSB = 13  # psum superblocks: 12 x 512 + 1 x 128
CHUNK = 32  # gather chunk size in K-tiles
NQ = 4  # SWDGE queues (round-robin; each runs on its own Q7 core pair)
DIMS = [(128, 128), (128, 128), (128, 64), (64, 128), (128, 128), (128, 128)]
RELU = [True, True, False, True, True, False]
TBLW = 128  # table row width (64-wide bottleneck zero-padded to 128, fp16)

import os as _os
DEBUG_NL = int(_os.environ.get("DEBUG_NL", "6"))  # layers to run (debug)
DEBUG_PREBN = int(_os.environ.get("DEBUG_PREBN", "0"))
DEBUG_REPS = int(_os.environ.get("DEBUG_REPS", "0"))
TRACE = False  # set by test.py for profiling runs
TRACE_KW = {}
LAST_RESULT = None  # BassKernelResults of the last run (for test.py)


def _prep_edges(src_remap, dstl):
    """Per-core edge prep: sort by dst; per-(window, half) edge lists."""
    order = np.argsort(dstl, kind="stable")
    dstl = dstl[order]
    srcr = src_remap[order]
    half = (srcr >= HALF).astype(np.int64)
    w = dstl // WIN
    rel = dstl - w * WIN
    ed = [[None, None] for _ in range(NWIN)]
    for wi in range(NWIN):
        m = w == wi
        for h in (0, 1):
            mh = m & (half == h)
            ed[wi][h] = (srcr[mh] - h * HALF, rel[mh])
    return ed


def _build_core_tables(ed, tiles):
    """Pack per-core edge lists into padded tile streams (per half)."""
    out = []
    for h in (0, 1):
        T = sum(tiles[w][h] for w in range(NWIN))
        gidx = np.zeros(T * 128, np.int16)
        drel = np.full(T * 128, -1.0, np.float32)
        t = 0
        for w in range(NWIN):
            g, r = ed[w][h]
            nt = tiles[w][h]
            assert len(g) <= nt * 128
            base = t * 128
            gidx[base : base + len(g)] = g.astype(np.int16)
            drel[base : base + len(g)] = r.astype(np.float32)
            t += nt
        out.append((gidx, drel))
    return out


def _wrap_idx(gidx, chunks):
    """int16 indices -> [128, total/16] wrapped per chunk, tiled 8x."""
    total_cols = len(gidx) // 16
    arr = np.zeros((16, total_cols), np.int16)
    col = 0
    for t0, nt in chunks:
        cidx = gidx[t0 * 128 : (t0 + nt) * 128]
        ncol = len(cidx) // 16
        arr[:, col : col + ncol] = cidx.reshape(ncol, 16).T
        col += ncol
    assert col == total_cols
    return np.tile(arr, (8, 1)).copy()


def _chunks_of(T):
    out = []
    t = 0
    while t < T:
        nt = min(CHUNK, T - t)
        out.append((t, nt))
        t += nt
    return out


def _build_program(tiles, t0s, TA, TB):
    from concourse import bacc, mybir, tile

    FP32 = mybir.dt.float32
    BF16 = mybir.dt.float16  # "BF16" alias now fp16: same perf, 8x finer mantissa
    I16 = mybir.dt.int16
    AX = mybir.AxisListType.X
    OP = mybir.AluOpType
    ACTF = mybir.ActivationFunctionType

    nc = bacc.Bacc(
        None,
        num_devices=NC,
        target_bir_lowering=False,
        debug=False,
        num_swdge_queues=NQ,
    )

    # ---- parameters ----
    HS0DT = BF16
    hs0_d = nc.declare_dram_parameter("hs0", [NP, F_IN], HS0DT, isOutput=False)
    hs0own_d = nc.declare_dram_parameter(
        "hs0own", [128, NWIN, F_IN], HS0DT, isOutput=False
    )
    idx_d = [
        nc.declare_dram_parameter("idxA", [128, TA * 8], I16, isOutput=False),
        nc.declare_dram_parameter("idxB", [128, TB * 8], I16, isOutput=False),
    ]
    dstl_d = [
        nc.declare_dram_parameter("dstlA", [128, TA], BF16, isOutput=False),
        nc.declare_dram_parameter("dstlB", [128, TB], BF16, isOutput=False),
    ]
    iota_d = nc.declare_dram_parameter("iota", [128, 128], BF16, isOutput=False)
    dinv_d = nc.declare_dram_parameter("dinvT", [128, SHP], FP32, isOutput=False)
    ident_d = nc.declare_dram_parameter("ident", [128, 128], FP32, isOutput=False)
    W_d = [
        nc.declare_dram_parameter(f"W{j}", list(DIMS[j]), FP32, isOutput=False)
        for j in range(6)
    ]
    gb_d = [
        nc.declare_dram_parameter(f"gb{j}", [128, 2], FP32, isOutput=False)
        for j in range(6)
    ]
    ones_d = nc.declare_dram_parameter("ones", [1, 128], FP32, isOutput=False)
    dinvnb_d = nc.declare_dram_parameter("dinvNB", [128, NWIN], FP32, isOutput=False)
    out_d = nc.declare_dram_parameter("out", [128, SHP], FP32, isOutput=True)

    # ---- internal DRAM: collective bounce buffers ----
    # table for layer j+1 is ag_out[j]; all fp16, 128-wide (64 zero-padded)
    ag_dt = [BF16 for j in range(5)]
    ag_in = [nc.dram_tensor(f"ag_in{j}", [SHP, TBLW], ag_dt[j]) for j in range(5)]
    ag_out = [
        nc.dram_tensor(f"ag_out{j}", [NP, TBLW], ag_dt[j], addr_space="Shared")
        for j in range(5)
    ]
    ar_in = [nc.dram_tensor(f"ar_in{j}", [128, 2], FP32) for j in range(6)]
    ar_out = [
        nc.dram_tensor(f"ar_out{j}", [128, 2], FP32, addr_space="Shared")
        for j in range(6)
    ]

    tbls = [hs0_d] + ag_out

    chunks = [_chunks_of(TA), _chunks_of(TB)]
    idx_col0 = [[], []]
    for h in (0, 1):
        c = 0
        for _, nt in chunks[h]:
            idx_col0[h].append(c)
            c += nt * 8

    with tile.TileContext(nc) as tc:
        with (
            tc.tile_pool(name="res", bufs=1) as res,
            tc.tile_pool(name="msg", bufs=8) as msgp,
            tc.tile_pool(name="sp", bufs=4) as sp,
            tc.tile_pool(name="small", bufs=2) as small,
            tc.tile_pool(name="big", bufs=1) as big,
            tc.tile_pool(name="hx", bufs=2) as hxp,
            tc.tile_pool(name="agg_ps", bufs=2, space="PSUM") as aggp,
            tc.tile_pool(name="y_ps", bufs=2, space="PSUM") as yp,
            tc.tile_pool(name="tr_ps", bufs=2, space="PSUM") as trp,
        ):
            # ---- resident loads ----
            idx_t = [res.tile([128, TA * 8], I16, name="idxA")]
            nc.sync.dma_start(idx_t[0][:], idx_d[0][:])
            idx_t.append(res.tile([128, TB * 8], I16, name="idxB"))
            nc.sync.dma_start(idx_t[1][:], idx_d[1][:])
            dstl_b = []
            for h, TH in ((0, TA), (1, TB)):
                db = res.tile([128, TH], BF16, name=f"dstl{h}")
                nc.sync.dma_start(db[:], dstl_d[h][:])
                dstl_b.append(db)
            iota_bt = res.tile([128, 128], BF16, name="iotab")
            nc.sync.dma_start(iota_bt[:], iota_d[:])
            ident_f = res.tile([128, 128], FP32, name="identf")
            nc.sync.dma_start(ident_f[:], ident_d[:])
            ident_b = res.tile([128, 128], BF16, name="identb")
            nc.vector.tensor_copy(ident_b[:], ident_f[:])
            ones_t = res.tile([1, 128], FP32, name="ones")
            nc.sync.dma_start(ones_t[:], ones_d[:])
            dinvnb_t = res.tile([128, NWIN], FP32, name="dinvnb")
            nc.sync.dma_start(dinvnb_t[:], dinvnb_d[:])
            W_t = []
            for j in range(6):
                wt = res.tile(list(DIMS[j]), FP32, name=f"W{j}")
                nc.sync.dma_start(wt[:], W_d[j][:])
                W_t.append(wt)
            gb_t = []
            for j in range(6):
                gt = res.tile([128, 2], FP32, name=f"gb{j}")
                nc.sync.dma_start(gt[:], gb_d[j][:])
                gb_t.append(gt)

            # own-shard tile (node-major; source for self-loop transposes)
            hprev = hxp.tile([128, NWIN, F_IN], HS0DT, tag="hx", name="hs0own")
            nc.sync.dma_start(hprev[:], hs0own_d[:])
            hprev_dt = HS0DT

            inv_n = 1.0 / float(N)
            qrr = [0]  # gather queue round-robin counter

            def bn_vec(j, fo, arr_tile):
                """mean/var -> (scale, shift) columns in a [128, 6] tile."""
                vec = small.tile([128, 6], FP32, tag="bnvec", name="vec")
                nc.vector.tensor_scalar(
                    out=vec[0:fo, 0:1], in0=arr_tile[0:fo, 0:1],
                    scalar1=inv_n, scalar2=None, op0=OP.mult,
                )
                nc.vector.tensor_scalar(
                    out=vec[0:fo, 1:2], in0=arr_tile[0:fo, 1:2],
                    scalar1=inv_n, scalar2=None, op0=OP.mult,
                )
                nc.vector.tensor_tensor(
                    vec[0:fo, 2:3], vec[0:fo, 0:1], vec[0:fo, 0:1], op=OP.mult
                )
                nc.vector.tensor_tensor(
                    vec[0:fo, 2:3], vec[0:fo, 1:2], vec[0:fo, 2:3],
                    op=OP.subtract,
                )
                nc.vector.tensor_scalar(
                    out=vec[0:fo, 2:3], in0=vec[0:fo, 2:3],
                    scalar1=float(EPS), scalar2=None, op0=OP.add,
                )
                nc.vector.reciprocal(vec[0:fo, 3:4], vec[0:fo, 2:3])
                nc.scalar.activation(vec[0:fo, 3:4], vec[0:fo, 3:4], ACTF.Sqrt)
                nc.vector.tensor_tensor(
                    vec[0:fo, 4:5], gb_t[j][0:fo, 0:1], vec[0:fo, 3:4],
                    op=OP.mult,
                )
                nc.vector.tensor_tensor(
                    vec[0:fo, 5:6], vec[0:fo, 0:1], vec[0:fo, 4:5], op=OP.mult
                )
                nc.vector.tensor_tensor(
                    vec[0:fo, 5:6], gb_t[j][0:fo, 1:2], vec[0:fo, 5:6],
                    op=OP.subtract,
                )
                return vec

            for j in range(DEBUG_NL):
                fi, fo = DIMS[j]
                tbl = tbls[j]
                MDT = BF16
                ident_in = ident_b if hprev_dt == BF16 else ident_f

                # emit all gather chunks up-front (async on 4 SWDGE queues);
                # msgp pool depth throttles them via buffer reuse.
                chunk_tiles = [[], []]
                order = []
                ka, kb = 0, 0
                while ka < len(chunks[0]) or kb < len(chunks[1]):
                    if ka < len(chunks[0]):
                        order.append((0, ka))
                        ka += 1
                    if kb < len(chunks[1]):
                        order.append((1, kb))
                        kb += 1
                for h, k in order:
                    t0c, ntc = chunks[h][k]
                    mt = msgp.tile([128, ntc, TBLW], MDT, tag="msg", name="msg")
                    nc.gpsimd.dma_gather(
                        out_ap=mt[:],
                        in_ap=tbl[h * HALF : (h + 1) * HALF, :],
                        idxs_ap=idx_t[h][
                            :, idx_col0[h][k] : idx_col0[h][k] + ntc * 8
                        ],
                        num_idxs=ntc * 128,
                        num_idxs_reg=ntc * 128,
                        elem_size=TBLW,
                        single_packet=False,
                        queue_num=qrr[0] % NQ,
                    )
                    qrr[0] += 1
                    chunk_tiles[h].append((t0c, ntc, mt))

                def msg_of(h, t):
                    for t0c, ntc, mt in chunk_tiles[h]:
                        if t0c <= t < t0c + ntc:
                            return t0c, mt
                    raise AssertionError((h, t))

                y_sb = big.tile([128, SHP], FP32, tag="ysb", name="ysb")
                sumP = small.tile([128, NSB], FP32, tag="sumP", name="sumP")
                sqP = small.tile([128, NSB], FP32, tag="sqP", name="sqP")
                junk = small.tile([128, 512], FP32, tag="junk", name="junk")

                for sb in range(NSB):
                    nsb = 512 if sb < 12 else 128
                    wlist = list(range(sb * 4, min(sb * 4 + 4, NWIN)))
                    # tile sequence; (w, None, None) = self-loop transpose
                    seq = []
                    for w in wlist:
                        seq.append((w, None, None))
                        for h in (0, 1):
                            for t in range(t0s[w][h], t0s[w][h] + tiles[w][h]):
                                seq.append((w, h, t))
                    # batched S build: one is_equal per half covering this
                    # sb's windows (dst one-hots from iota/dstl broadcast)
                    st_h = [None, None]
                    st_base = [0, 0]
                    for h in (0, 1):
                        tb0 = t0s[wlist[0]][h]
                        tb1 = t0s[wlist[-1]][h] + tiles[wlist[-1]][h]
                        st_base[h] = tb0
                        if tb1 > tb0:
                            G = tb1 - tb0
                            st = sp.tile([128, G, 128], MDT, tag="S", name="S")
                            i0 = iota_bt[:].rearrange(
                                "p (g d) -> p g d", g=1
                            ).broadcast_to([128, G, 128])
                            i1 = dstl_b[h][:, tb0:tb1].broadcast_to(
                                [128, G, 128]
                            )
                            nc.vector.tensor_tensor(
                                st[:], i0, i1, op=OP.is_equal
                            )
                            st_h[h] = st
                    agg = aggp.tile([128, 512], FP32, tag="agg", name="agg")
                    for i, (w, h, t) in enumerate(seq):
                        woff = (w % 4) * 128
                        first = i == 0
                        last = i == len(seq) - 1
                        if h is None:
                            # self-loop: agg[f, d] += hprev[d, f], done as a
                            # regular matmul with identity moving operand
                            # (lhsT=hprev) so fp16 input can hit f32 PSUM.
                            nc.tensor.matmul(
                                agg[0:fi, woff : woff + 128],
                                hprev[:, w, 0:fi],
                                ident_in[:],
                                start=first,
                                stop=last,
                            )
                            continue
                        t0c, mt = msg_of(h, t)
                        nc.tensor.matmul(
                            agg[0:fi, woff : woff + 128],
                            mt[:, t - t0c, 0:fi],
                            st_h[h][:, t - st_base[h], :],
                            start=first,
                            stop=last,
                        )
                    # evict + dinv[dst] scale
                    dv = small.tile([128, 512], FP32, tag="dinv", name="dv")
                    nc.sync.dma_start(
                        dv[:, 0:nsb], dinv_d[:, sb * 512 : sb * 512 + nsb]
                    )
                    rawT = small.tile([128, 512], FP32, tag="rawT", name="rawT")
                    nc.vector.tensor_tensor(
                        rawT[0:fi, 0:nsb],
                        agg[0:fi, 0:nsb],
                        dv[0:fi, 0:nsb],
                        op=OP.mult,
                    )
                    # W matmul (f32)
                    y_ps = yp.tile([128, 512], FP32, tag="yps", name="yps")
                    nc.tensor.matmul(
                        y_ps[0:fo, 0:nsb],
                        W_t[j][:],
                        rawT[0:fi, 0:nsb],
                        start=True,
                        stop=True,
                    )
                    # copy to y_sb + stats over valid columns
                    nv = 512 if sb < 12 else 106
                    c0 = sb * 512
                    nc.scalar.activation(
                        y_sb[0:fo, c0 : c0 + nv],
                        y_ps[0:fo, 0:nv],
                        ACTF.Copy,
                        accum_out=sumP[0:fo, sb : sb + 1],
                    )
                    if sb == 12:
                        nc.scalar.activation(
                            y_sb[0:fo, c0 + 106 : c0 + 128],
                            y_ps[0:fo, 106:128],
                            ACTF.Copy,
                        )
                    nc.scalar.activation(
                        junk[0:fo, 0:nv],
                        y_ps[0:fo, 0:nv],
                        ACTF.Square,
                        accum_out=sqP[0:fo, sb : sb + 1],
                    )

                # ---- kick BN stats all-reduce ----
                stats = small.tile([128, 2], FP32, tag="stats", name="stats")
                nc.vector.memset(stats[:], 0.0)
                nc.vector.reduce_sum(stats[0:fo, 0:1], sumP[0:fo, :], axis=AX)
                nc.vector.reduce_sum(stats[0:fo, 1:2], sqP[0:fo, :], axis=AX)
                nc.sync.dma_start(ar_in[j][:], stats[:])
                nc.gpsimd.collective_compute(
                    "AllReduce",
                    OP.add,
                    replica_groups=[list(range(NC))],
                    ins=[ar_in[j][:]],
                    outs=[ar_out[j][:]],
                )

                if j == DEBUG_NL - 1 and j != 5:
                    pass  # fall through; debug dump happens below
                if j == 5:
                    # final layer: BN in feat-major via ACT, DMA out
                    arr = small.tile([128, 2], FP32, tag="arr", name="arr")
                    nc.sync.dma_start(arr[:], ar_out[j][:])
                    vec = bn_vec(j, fo, arr)
                    for sb in range(NSB):
                        nsb = 512 if sb < 12 else 128
                        c0 = sb * 512
                        nc.scalar.activation(
                            y_sb[0:fo, c0 : c0 + nsb],
                            y_sb[0:fo, c0 : c0 + nsb],
                            ACTF.Identity,
                            bias=vec[0:fo, 5:6],
                            scale=vec[0:fo, 4:5],
                        )
                    nc.sync.dma_start(out_d[:], y_sb[:])
                    continue

                # ---- transpose raw y to node-major (overlaps the AR) ----
                hnext = hxp.tile([128, NWIN, TBLW], ag_dt[j], tag="hx", name="hnext")
                if fo < TBLW:
                    nc.vector.memset(hnext[:, :, fo:TBLW], 0.0)
                for b0 in range(0, NWIN, 4):
                    nb = min(4, NWIN - b0)
                    tr4 = trp.tile([128, 512], FP32, tag="tr", name="tr4")
                    for bi in range(nb):
                        b = b0 + bi
                        nc.tensor.matmul(
                            tr4[0:128, bi * fo : bi * fo + fo],
                            y_sb[0:fo, b * 128 : (b + 1) * 128],
                            ident_f[0:fo, 0:fo],
                            is_transpose=True,
                            start=True,
                            stop=True,
                        )
                    nc.vector.tensor_copy(
                        hnext[:, b0 : b0 + nb, 0:fo],
                        tr4[:, 0 : nb * fo].rearrange("p (b f) -> p b f", f=fo),
                    )

                # ---- AR result -> scale/shift -> rank-1 broadcast tiles ----
                arr = small.tile([128, 2], FP32, tag="arr", name="arr")
                nc.sync.dma_start(arr[:], ar_out[j][:])
                vec = bn_vec(j, fo, arr)
                # transpose scale/shift columns to rows, then rank-1 bcast
                scale_rep = small.tile(
                    [128, 128], ag_dt[j], tag="srep", name="scale_rep"
                )
                shift_rep = small.tile(
                    [128, 128], ag_dt[j], tag="hrep", name="shift_rep"
                )
                for col, rep in ((4, scale_rep), (5, shift_rep)):
                    vt_ps = trp.tile([128, 128], FP32, tag="tr", name="vtps")
                    nc.tensor.matmul(
                        vt_ps[0:1, 0:fo],
                        vec[0:fo, col : col + 1],
                        ident_f[0:fo, 0:fo],
                        is_transpose=True,
                        start=True,
                        stop=True,
                    )
                    vrow = small.tile([1, 128], FP32, tag="vrow", name="vrow")
                    nc.vector.tensor_copy(vrow[:, 0:fo], vt_ps[0:1, 0:fo])
                    rep_ps = trp.tile([128, 128], FP32, tag="tr", name="repps")
                    nc.tensor.matmul(
                        rep_ps[:, 0:fo], ones_t[:], vrow[0:1, 0:fo],
                        start=True, stop=True,
                    )
                    nc.vector.tensor_copy(rep[:, 0:fo], rep_ps[:, 0:fo])

                if DEBUG_REPS and j == DEBUG_NL - 1:
                    nc.sync.dma_start(out_d[:, 0:128], scale_rep[:])
                    nc.sync.dma_start(out_d[:, 128:256], shift_rep[:])
                    nc.sync.dma_start(out_d[:, 256:262], vec[:, 0:6])
                    continue
                # ---- BN apply (+ReLU) in node-major, in place ----
                for b in range(NWIN if not (DEBUG_PREBN and j == DEBUG_NL - 1) else 0):
                    blk = hnext[:, b, 0:fo]
                    nc.vector.tensor_tensor(
                        blk, blk, scale_rep[:, 0:fo], op=OP.mult
                    )
                    nc.vector.tensor_tensor(
                        blk, blk, shift_rep[:, 0:fo], op=OP.add
                    )
                    if RELU[j]:
                        nc.vector.tensor_scalar(
                            out=blk, in0=blk,
                            scalar1=dinvnb_t[:, b : b + 1], scalar2=0.0,
                            op0=OP.mult, op1=OP.max,
                        )
                    else:
                        nc.vector.tensor_scalar(
                            out=blk, in0=blk,
                            scalar1=dinvnb_t[:, b : b + 1], scalar2=None,
                            op0=OP.mult,
                        )

                if j == DEBUG_NL - 1:
                    nc.sync.dma_start(
                        out_d[:, 0 : NWIN * fo].rearrange(
                            "p (b f) -> p b f", f=fo
                        ),
                        hnext[:, :, 0:fo],
                    )
                    continue
                nc.sync.dma_start(
                    ag_in[j][:].rearrange("(b p) f -> p b f", p=128), hnext[:]
                )
                nc.gpsimd.collective_compute(
                    "AllGather",
                    OP.bypass,
                    replica_groups=[list(range(NC))],
                    ins=[ag_in[j][:]],
                    outs=[ag_out[j][:]],
                )
                hprev = hnext
                hprev_dt = ag_dt[j]

    nc.compile()
    return nc


def kernel(x, edge_index, **params):
    global LAST_RESULT
    import ml_dtypes

    from concourse.bass_utils import run_bass_kernel_spmd

    x = np.asarray(x, np.float32)
    edge_index = np.asarray(edge_index, np.int64)
    src_all = edge_index[0]
    dst_all = edge_index[1]

    deg = (np.bincount(dst_all, minlength=N) + 1.0).astype(np.float32)
    dinv = (1.0 / np.sqrt(deg)).astype(np.float32)

    hs0 = np.zeros((NP, F_IN), np.float32)
    xs = x * dinv[:, None]
    for c in range(NC):
        hs0[c * SHP : c * SHP + SH] = xs[c * SH : (c + 1) * SH]
    hs0_bf = hs0.astype(np.float16)

    remap = (src_all // SH) * SHP + (src_all % SH)

    eds = []
    for c in range(NC):
        m = (dst_all >= c * SH) & (dst_all < (c + 1) * SH)
        dstl = dst_all[m] - c * SH
        srcr = remap[m]
        eds.append(_prep_edges(srcr, dstl))

    tiles = [[0, 0] for _ in range(NWIN)]
    for w in range(NWIN):
        for h in (0, 1):
            mx = max(len(eds[c][w][h][0]) for c in range(NC))
            tiles[w][h] = -(-mx // 128) if mx else 0
    t0s = [[0, 0] for _ in range(NWIN)]
    ta = tb = 0
    for w in range(NWIN):
        t0s[w][0] = ta
        ta += tiles[w][0]
        t0s[w][1] = tb
        tb += tiles[w][1]
    TA, TB = ta, tb

    chunksA = _chunks_of(TA)
    chunksB = _chunks_of(TB)

    in_maps = []
    for c in range(NC):
        (gA, dA), (gB, dB) = _build_core_tables(eds[c], tiles)
        dinvT = np.zeros(SHP, np.float32)
        dinvT[:SH] = dinv[c * SH : (c + 1) * SH]
        # own shard in [128, NWIN, F] node-major layout: [p, b, :] = node b*128+p
        own = hs0_bf[c * SHP : (c + 1) * SHP].reshape(NWIN, 128, F_IN)
        im = {
            "hs0": hs0_bf,
            "hs0own": np.ascontiguousarray(own.transpose(1, 0, 2)),
            "idxA": _wrap_idx(gA, chunksA),
            "idxB": _wrap_idx(gB, chunksB),
            "dstlA": dA.reshape(TA, 128).T.astype(np.float16).copy(),
            "dstlB": dB.reshape(TB, 128).T.astype(np.float16).copy(),
            "iota": np.broadcast_to(
                np.arange(128, dtype=np.float16), (128, 128)
            ).copy(),
            "dinvT": np.broadcast_to(dinvT, (128, SHP)).copy(),
            "ident": np.eye(128, dtype=np.float32),
            "ones": np.ones((1, 128), np.float32),
            "dinvNB": np.ascontiguousarray(
                dinvT.reshape(NWIN, 128).T
            ),
        }
        for j in range(6):
            im[f"W{j}"] = np.asarray(params[f"W{j}"], np.float32)
            gb = np.zeros((128, 2), np.float32)
            fo = DIMS[j][1]
            gb[:fo, 0] = np.asarray(params[f"g{j}"], np.float32)
            gb[:fo, 1] = np.asarray(params[f"be{j}"], np.float32)
            im[f"gb{j}"] = gb
        in_maps.append(im)

    nc = _build_program(tiles, t0s, TA, TB)
    res = run_bass_kernel_spmd(
        nc,
        in_maps,
        core_ids=list(range(NC)),
        trace=TRACE,
        **TRACE_KW,
    )
    LAST_RESULT = res

    out = np.empty((N, F_IN), np.float32)
    for c in range(NC):
        out[c * SH : (c + 1) * SH] = res.results[c]["out"].T[:SH]
    return out

